# revision 1
# baseline (speedup 1.0000x reference)
"""Trainium2 Bass kernel for the binarized BasicBlock (dense_cnn).

Contract: kernel(**inputs) takes the FULL unsharded inputs (numpy arrays,
keyed as in reference.setup_inputs()) and returns the FULL output
(32, 128, 56, 56) float32.  Internally shards the batch dim across 8
NeuronCores (pure data parallel, params replicated).

Per-core layout: 4 images processed as 2 pairs; each pair in 2 half-height
units of 28 output rows.  Partitions hold (imgA ch0-63 | imgB ch0-63) for
stage-1 tensors.  Conv1 runs as 9 shifted matmuls per psum chunk with images
A/B on concurrent 64x64 PE tiles; avgpool shortcut on DVE in fp32 (exact, so
sign2 never flips); PReLU stages are single ACT Prelu ops reading PSUM with
per-partition scale/bias/alpha; stage-2 residual is injected into PSUM via a
diag matmul of bf16(out1), with the diag/scale pair rounding-compensated.
"""
import sys

sys.path.insert(0, "/opt/trn_rl_repo")

import numpy as np
import ml_dtypes

import concourse.bacc as bacc
import concourse.mybir as mybir
import concourse.tile as tile
from concourse import bass_utils

# Problem shapes (hardcoded per spec)
B, CIN, H, W = 32, 64, 112, 112
COUT = 2 * CIN
NCORES = 8
BPC = B // NCORES          # images per core = 4
NPAIR = BPC // 2           # image pairs per core = 2
OH, OW = H // 2, W // 2    # 56, 56
HALF = OH // 2             # 28 output rows per unit
NCHUNK = 4                 # psum chunks per unit (7 out rows each)
CROWS = HALF // NCHUNK     # 7
CN = CROWS * OW            # 392 cols per chunk
UN = HALF * OW             # 1568 elems per unit (per partition)
SROWS = 57                 # raw/sign slab rows (input rows 2*oy0-1 .. 2*oy0+55)
SPITCH = 114               # sign slab col pitch (1 left pad + 112 + 1 right pad)

# param columns
PA1, PB12, PB11, PA2F, PB22F, PS2V, PBS2, PB13, PB23F = range(9)
NPARAM = 9
# weight blocks of 64 cols: conv taps 0..8 (ky*3+kx); then two 128-wide
# blocks: [wpw1|wpw2] and [diag1|diag2] for M=128 stage-2 matmuls
NBLK = 9
WCOLS = NBLK * 64 + 256
O_PW = NBLK * 64          # [wpw1|wpw2] at cols O_PW:O_PW+128
O_DIAG = NBLK * 64 + 128  # [diag1|diag2]

_cache = {}


def _build(scal, reps=1):
    """Build the bass program. scal: host-derived scalars/flags.
    reps>1 replicates the whole compute (for slope-based device timing)."""
    nc = bacc.Bacc("TRN2", target_bir_lowering=False, debug=False)
    f32 = mybir.dt.float32
    bf16 = mybir.dt.bfloat16
    u32 = mybir.dt.uint32
    AF = mybir.ActivationFunctionType
    ALU = mybir.AluOpType

    s3x4 = scal["s3x4"]
    fast_sign2 = scal["fast_sign2"]
    sign1_gpsimd = scal["sign1_gpsimd"]
    has_b13 = scal["has_b13"]
    has_b23 = scal["has_b23"]

    tc_cm = tile.TileContext(nc)
    tc = tc_cm.__enter__()
    dram_cm = tc.tile_pool(name="dram", bufs=1, space="DRAM")
    dram = dram_cm.__enter__()

    x_d = dram.tile([BPC, CIN, H, W], f32, kind="ExternalInput")
    w_d = dram.tile([128, WCOLS], bf16, kind="ExternalInput")
    p_d = dram.tile([128, NPARAM], f32, kind="ExternalInput")
    y_d = dram.tile([BPC, COUT, OH, OW], f32, kind="ExternalOutput")

    pools = []

    def pool(name, **kw):
        cm = tc.tile_pool(name=name, **kw)
        pools.append(cm)
        return cm.__enter__()

    const = pool("const", bufs=1)
    pers = pool("pers", bufs=1)
    work = pool("work", bufs=2)
    work1 = pool("work1", bufs=1)
    psum = pool("psum", bufs=4, space="PSUM")

    wt = const.tile([128, WCOLS], bf16)
    pt = const.tile([128, NPARAM], f32)
    nc.sync.dma_start(wt[:], w_d[:])
    nc.sync.dma_start(pt[:], p_d[:])

    # persistent slabs: index by half h (stable pad semantics per buffer)
    xp = [pers.tile([128, SROWS * W], f32, tag=f"xp{h}", name=f"xp{h}")
          for h in range(2)]
    sp = [pers.tile([128, SROWS * SPITCH], bf16, tag=f"sp{h}", name=f"sp{h}")
          for h in range(2)]
    for h in range(2):
        # zero only the pad borders (row 0, col 0, col 113)
        spv0 = sp[h][:].rearrange("p (r c) -> p r c", r=SROWS)
        nc.vector.memset(spv0[:, 0:1, :], 0.0)
        nc.vector.memset(spv0[:, :, 0:1], 0.0)
        nc.vector.memset(spv0[:, :, 113:114], 0.0)

    def wap(blk):
        # lhsT view for block blk: [128, 64]; callers slice partition range
        return wt[:, 64 * blk:64 * blk + 64]

    units = [(p, h) for _ in range(reps)
             for p in range(NPAIR) for h in range(2)]
    s4s = {}

    def emit_a(k):
        """Phase A of unit k: x load, sign1 -> sp, avgpool -> s4."""
        if k >= len(units):
            return
        p, h = units[k]
        nA = 2 * p
        oy0 = HALF * h
        r0 = 2 * oy0 - 1           # input row of slab row 0
        ld0 = 1 if h == 0 else 0   # first valid slab row
        nrows = SROWS - ld0        # rows loaded
        in0 = r0 + ld0             # first input row loaded

        xpv = xp[h][:].rearrange("p (r c) -> p r c", r=SROWS)
        spv = sp[h][:].rearrange("p (r c) -> p r c", r=SROWS)

        # k==0: band-split load+sign1 so the first conv starts early
        bands = ([(ld0, 15), (15, 29), (29, 43), (43, SROWS)] if k == 0
                 else [(ld0, SROWS)])
        for (ra, rb) in bands:
            src = x_d[nA:nA + 2, :, r0 + ra:r0 + rb, :].rearrange(
                "i c r w -> (i c) r w")
            nc.sync.dma_start(xpv[:, ra:rb, :], src)
            if k == 0 or not sign1_gpsimd:
                nc.scalar.activation(
                    spv[:, ra:rb, 1:113], xpv[:, ra:rb, :],
                    AF.Sign, bias=pt[:, PB11:PB11 + 1])
        if k > 0 and sign1_gpsimd:
            # split ACT / DVE to balance engines
            na = ld0 + 38          # ACT rows [ld0, na); DVE rows [na, 57)
            nc.scalar.activation(
                spv[:, ld0:na, 1:113], xpv[:, ld0:na, :], AF.Sign)
            s1f = work1.tile([128, SROWS * W], f32, tag="s1f", name="s1f")
            flat = slice(na * W, SROWS * W)
            nc.vector.tensor_scalar(
                s1f[:, flat].bitcast(u32), xp[h][:, flat].bitcast(u32),
                0x80000000, 0x3F800000,
                ALU.bitwise_and, ALU.bitwise_or)
            s1v = s1f[:].rearrange("p (r c) -> p r c", r=SROWS)
            nc.vector.tensor_copy(spv[:, na:SROWS, 1:113],
                                  s1v[:, na:SROWS, :])

        # avgpool x4 on DVE (fp32 exact)
        prow = work1.tile([128, HALF * W], f32, tag="prow", name="prow")
        prv = prow[:].rearrange("p (r c) -> p r c", r=HALF)
        nc.vector.tensor_tensor(
            prv[:], xpv[:, 1:SROWS:2, :], xpv[:, 2:SROWS:2, :], ALU.add)
        s4 = work.tile([128, UN], f32, tag="s4", name="s4")
        s4v = s4[:].rearrange("p (r c) -> p r c", r=HALF)
        nc.vector.tensor_tensor(
            s4v[:], prv[:, :, 0:W:2], prv[:, :, 1:W:2], ALU.add)
        s4s[k] = s4

    emit_a(0)
    for k, (p, h) in enumerate(units):
        nA, nB = 2 * p, 2 * p + 1
        oy0 = HALF * h
        s4 = s4s.pop(k)
        spv = sp[h][:].rearrange("p (r c) -> p r c", r=SROWS)

        # ---- conv1: 9 taps x 4 chunks, A/B on concurrent 64x64 tiles ----
        u = work.tile([128, UN], f32, tag="u", name="u")
        for c in range(NCHUNK):
            cpAB = [psum.tile([128, CN], f32, tag=f"ps{i}", name=f"ps{i}")
                    for i in range(2)]
            for t in range(9):
                ky, kx = divmod(t, 3)
                rs = ky + 14 * c
                for i in range(2):
                    pr = slice(64 * i, 64 * i + 64)
                    rhs = spv[pr, rs:rs + 13:2, kx:kx + 111:2]
                    nc.tensor.matmul(
                        cpAB[i][pr, :], wap(t)[pr, :], rhs,
                        start=(t == 0), stop=(t == 8),
                    )
            # u_c = 4*s3*conv + S4  (fused scalar_tensor_tensor)
            cs = slice(CN * c, CN * (c + 1))
            for i in range(2):
                pr = slice(64 * i, 64 * i + 64)
                nc.vector.scalar_tensor_tensor(
                    u[pr, cs], cpAB[i][pr, :], s3x4, s4[pr, cs],
                    ALU.mult, ALU.add)

        # hoist next unit's load/sign1/pool: its ACT/DVE/DMA work overlaps
        # this unit's conv matmuls and stage-2
        emit_a(k + 1)

        # ---- prelu1 (-> bf16 out1) / sign2, per chunk ----
        out1 = work.tile([128, UN], bf16, tag="out1", name="out1")
        sg2 = work.tile([128, UN], bf16, tag="sg2", name="sg2")
        for c in range(NCHUNK):
            cs = slice(CN * c, CN * (c + 1))
            nc.scalar.activation(
                out1[:, cs], u[:, cs], AF.Prelu,
                bias=pt[:, PB12:PB12 + 1], scale=0.25,
                alpha=pt[:, PA1:PA1 + 1])
            if fast_sign2:
                nc.scalar.activation(
                    sg2[:, cs], u[:, cs], AF.Sign,
                    bias=pt[:, PB12:PB12 + 1], scale=0.25)
        if has_b13:
            nc.vector.tensor_scalar(
                out1[:], out1[:], pt[:, PB13:PB13 + 1], None, ALU.add)
        if not fast_sign2:
            nc.scalar.activation(
                sg2[:], out1[:], AF.Sign, bias=pt[:, PBS2:PBS2 + 1])

        # ---- stage 2: per-image psum = (o1 | o2), residual injected ----
        # M=128 matmuls: lhsT [64, 128] = [wpw1|wpw2] then [diag1|diag2]
        stg = [work.tile([128, UN], f32, tag=f"stg{i}", name=f"stg{i}")
               for i in range(2)]
        for i, n in enumerate((nA, nB)):
            pr = slice(64 * i, 64 * i + 64)   # rhs partitions (image i)
            for c in range(NCHUNK):
                cp = psum.tile([128, CN], f32, tag=f"ps{i}", name=f"ps{i}")
                cs = slice(CN * c, CN * (c + 1))
                nc.tensor.matmul(
                    cp[:], wt[pr, O_PW:O_PW + 128], sg2[pr, cs],
                    start=True, stop=False)
                nc.tensor.matmul(
                    cp[:], wt[pr, O_DIAG:O_DIAG + 128], out1[pr, cs],
                    start=False, stop=True)
                nc.scalar.activation(
                    stg[i][:, cs], cp[:], AF.Prelu,
                    bias=pt[:, PB22F:PB22F + 1],
                    scale=pt[:, PS2V:PS2V + 1],
                    alpha=pt[:, PA2F:PA2F + 1])
            if has_b23:
                nc.vector.tensor_scalar(
                    stg[i][:], stg[i][:], pt[:, PB23F:PB23F + 1],
                    None, ALU.add)

        # ---- store: two 128-partition DMAs per image (overlap tail) ----
        for i, n in enumerate((nA, nB)):
            sv = stg[i][:].rearrange("p (r c) -> p r c", r=HALF)
            hh = HALF // 2
            nc.sync.dma_start(y_d[n, :, oy0:oy0 + hh, :], sv[:, 0:hh, :])
            nc.sync.dma_start(y_d[n, :, oy0 + hh:oy0 + HALF, :],
                              sv[:, hh:HALF, :])

    for cm in reversed(pools):
        cm.__exit__(None, None, None)
    dram_cm.__exit__(None, None, None)
    tc_cm.__exit__(None, None, None)
    nc.compile()
    return nc, x_d.name, w_d.name, p_d.name, y_d.name


def _prep(inputs):
    f32 = np.float32
    bf = ml_dtypes.bfloat16
    w3 = np.asarray(inputs["w3"], f32)
    wpw1 = np.asarray(inputs["wpw1"], f32)
    wpw2 = np.asarray(inputs["wpw2"], f32)
    a1 = np.asarray(inputs["a1"], f32).reshape(CIN)
    a2 = np.asarray(inputs["a2"], f32).reshape(COUT)
    b11 = np.asarray(inputs["b11"], f32).reshape(CIN)
    b12 = np.asarray(inputs["b12"], f32).reshape(CIN)
    b13 = np.asarray(inputs["b13"], f32).reshape(CIN)
    b21 = np.asarray(inputs["b21"], f32).reshape(CIN)
    b22 = np.asarray(inputs["b22"], f32).reshape(COUT)
    b23 = np.asarray(inputs["b23"], f32).reshape(COUT)

    s3 = float(np.mean(np.abs(w3))) or 1.0
    s1 = float(np.mean(np.abs(wpw1))) or 1.0
    s2 = float(np.mean(np.abs(wpw2))) or 1.0

    # diag entries bf16(1/s_j); prelu2 scale 1/d_j compensates the rounding
    d1 = float(bf(1.0 / s1))
    d2 = float(bf(1.0 / s2))

    whalf = np.zeros((64, WCOLS), f32)
    sgn = np.sign
    for t in range(9):
        ky, kx = divmod(t, 3)
        whalf[:, 64 * t:64 * t + 64] = sgn(w3[:, :, ky, kx]).T
    whalf[:, O_PW:O_PW + 64] = sgn(wpw1[:, :, 0, 0]).T
    whalf[:, O_PW + 64:O_PW + 128] = sgn(wpw2[:, :, 0, 0]).T
    whalf[:, O_DIAG:O_DIAG + 64] = d1 * np.eye(64, dtype=f32)
    whalf[:, O_DIAG + 64:O_DIAG + 128] = d2 * np.eye(64, dtype=f32)
    wfull = np.concatenate([whalf, whalf], axis=0).astype(bf)

    def pairc(v):  # channel vec (64,) -> pair-layout (128,)
        return np.concatenate([v, v])

    params = np.zeros((128, NPARAM), f32)
    params[:, PA1] = pairc(a1)
    params[:, PB12] = pairc(b12)
    params[:, PB11] = pairc(b11)
    params[:, PA2F] = a2
    params[:, PB22F] = b22
    params[:, PS2V] = np.concatenate(
        [np.full(64, 1.0 / d1, f32), np.full(64, 1.0 / d2, f32)])
    params[:, PBS2] = pairc(b13 + b21)
    params[:, PB13] = pairc(b13)
    params[:, PB23F] = b23

    scal = {
        "s3x4": 4.0 * s3,
        "fast_sign2": bool(np.all(b13 + b21 == 0.0) and np.all(a1 > 0)),
        "sign1_gpsimd": bool(np.all(b11 == 0.0)),
        "has_b13": bool(np.any(b13 != 0.0)),
        "has_b23": bool(np.any(b23 != 0.0)),
    }
    return wfull, params, scal


def kernel(**inputs):
    x = np.ascontiguousarray(np.asarray(inputs["x"], np.float32))
    wfull, params, scal = _prep(inputs)

    key = tuple(sorted(scal.items())) + (float(params.sum()),)
    if key not in _cache:
        _cache.clear()
        _cache[key] = _build(scal)
    nc, xn, wn, pn, yn = _cache[key]

    in_maps = []
    for i in range(NCORES):
        in_maps.append({
            xn: np.ascontiguousarray(x[BPC * i:BPC * (i + 1)]),
            wn: wfull,
            pn: params,
        })
    res = bass_utils.run_bass_kernel_spmd(nc, in_maps, core_ids=list(range(NCORES)))
    out = np.concatenate([res.results[i][yn] for i in range(NCORES)], axis=0)
    return out.astype(np.float32)



# revision 4
# speedup vs baseline: 193.6593x; 193.6593x over previous
"""Trainium2 Bass kernel for the binarized BasicBlock (dense_cnn).

Contract: kernel(**inputs) takes the FULL unsharded inputs (numpy arrays,
keyed as in reference.setup_inputs()) and returns the FULL output
(32, 128, 56, 56) float32.  Internally shards the batch dim across 8
NeuronCores (pure data parallel, params replicated).

Per-core layout: 4 images as 2 pairs; each pair in 2 half-height units of
28 output rows.  Partitions hold (imgA ch0-63 | imgB ch0-63).  The binary
3x3 conv runs as fp8 DoubleRow matmuls: the stride-2 conv reads adjacent
sign-slab columns (2ox+kx, 2ox+kx+1), so taps (ky,0)+(ky,1) pair naturally
as the two DoubleRow K-planes and (ky,2) pairs with a zero plane -- 6 DR
matmuls per 7-row chunk compute both images into one PSUM tile via
block-diagonal weights.  The avgpool shortcut stays exact fp32 (DVE/GPSIMD
adds) so sign2 never flips vs the reference; sign1 is split between ACT
(AF.Sign -> fp8) and DVE (u32 bitwise trick + fp8 cast).  Stage-2 is the
proven bf16 path: [wpw1|wpw2] pw matmul + diag residual injection into
PSUM, PReLU with per-partition rounding-compensated scales.  Outputs are
stored bf16 and upcast on the host, halving store traffic.
"""
import sys

sys.path.insert(0, "/opt/trn_rl_repo")

import numpy as np
import ml_dtypes

import concourse.bacc as bacc
import concourse.mybir as mybir
import concourse.tile as tile
from concourse import bass_utils

# Problem shapes (hardcoded per spec)
B, CIN, H, W = 32, 64, 112, 112
COUT = 2 * CIN
NCORES = 8
BPC = B // NCORES          # images per core = 4
NPAIR = BPC // 2           # image pairs per core = 2
OH, OW = H // 2, W // 2    # 56, 56
HALF = OH // 2             # 28 output rows per unit
NCHUNK = 4                 # psum chunks per unit (7 out rows each)
CROWS = HALF // NCHUNK     # 7
CN = CROWS * OW            # 392 cols per chunk
UN = HALF * OW             # 1568 elems per unit (per partition)
SROWS = 57                 # raw/sign slab rows (input rows 2*oy0-1 .. 2*oy0+55)
SPITCH = 114               # sign slab col pitch (1 left pad + 112 + 1 right pad)
NA_ACT = 30                # sign1 slab rows on ACT; rest on DVE (balance)

# param columns
PA1, PB12, PB11, PA2F, PB22F, PS2V, PBS2, PB13, PB23F, PS1 = range(10)
NPARAM = 10
# fp8 DoubleRow conv weight blocks: 6 blocks of [128, 2*128]
#   blocks 0..2: planes (ky,0),(ky,1) for ky=0,1,2
#   blocks 3..5: planes (ky,2),zero  for ky=0,1,2
NDR = 6
DRW = 2 * 128
# bf16 stage-2 weights: [wpw1|wpw2] then [diag1|diag2]
O_PW = 0
O_DIAG = 128
W2COLS = 256

_cache = {}


def _build(scal, reps=1):
    """Build the bass program. scal: host-derived scalars/flags.
    reps>1 replicates the whole compute (for slope-based device timing)."""
    nc = bacc.Bacc("TRN2", target_bir_lowering=False, debug=False)
    f32 = mybir.dt.float32
    bf16 = mybir.dt.bfloat16
    fp8 = mybir.dt.float8e4
    u32 = mybir.dt.uint32
    AF = mybir.ActivationFunctionType
    ALU = mybir.AluOpType

    fast_sign2 = scal["fast_sign2"]
    sign1_bitwise = scal["sign1_bitwise"]
    has_b13 = scal["has_b13"]
    has_b23 = scal["has_b23"]

    tc_cm = tile.TileContext(nc)
    tc = tc_cm.__enter__()
    dram_cm = tc.tile_pool(name="dram", bufs=1, space="DRAM")
    dram = dram_cm.__enter__()

    x_d = dram.tile([BPC, CIN, H, W], f32, kind="ExternalInput")
    wdr_d = dram.tile([128, NDR * DRW], fp8, kind="ExternalInput")
    w2_d = dram.tile([128, W2COLS], bf16, kind="ExternalInput")
    p_d = dram.tile([128, NPARAM], f32, kind="ExternalInput")
    y_d = dram.tile([BPC, COUT, OH, OW], bf16, kind="ExternalOutput")

    pools = []

    def pool(name, **kw):
        cm = tc.tile_pool(name=name, **kw)
        pools.append(cm)
        return cm.__enter__()

    const = pool("const", bufs=1)
    pers = pool("pers", bufs=1)
    work = pool("work", bufs=2)
    work1 = pool("work1", bufs=1)
    psum = pool("psum", bufs=4, space="PSUM")
    psum2 = pool("psum2", bufs=2, space="PSUM")

    wdr = const.tile([128, NDR * DRW], fp8)
    w2 = const.tile([128, W2COLS], bf16)
    pt = const.tile([128, NPARAM], f32)
    nc.sync.dma_start(wdr[:], wdr_d[:])
    nc.sync.dma_start(w2[:], w2_d[:])
    nc.sync.dma_start(pt[:], p_d[:])

    # persistent slabs: index by half h (stable pad semantics per buffer)
    xp = [pers.tile([128, SROWS * W], f32, tag=f"xp{h}", name=f"xp{h}")
          for h in range(2)]
    sp = [pers.tile([128, SROWS * SPITCH], fp8, tag=f"sp{h}", name=f"sp{h}")
          for h in range(2)]
    for h in range(2):
        # zero only the pad borders (row 0, col 0, cols 112..113 are data or
        # right pad; col 113 is the DoubleRow zero-plane read for ox=55)
        spv0 = sp[h][:].rearrange("p (r c) -> p r c", r=SROWS)
        nc.vector.memset(spv0[:, 0:1, :], 0.0)
        nc.vector.memset(spv0[:, :, 0:1], 0.0)
        nc.vector.memset(spv0[:, :, 113:114], 0.0)

    def drw(blk):
        # DoubleRow lhsT view for block blk: [128, 2, 128]
        return wdr[:, DRW * blk:DRW * blk + DRW].rearrange(
            "p (two m) -> p two m", two=2)

    units = [(p, h) for _ in range(reps)
             for p in range(NPAIR) for h in range(2)]
    s4s = {}

    def emit_a(k):
        """Phase A of unit k: x load, sign1 -> sp, avgpool -> s4."""
        if k >= len(units):
            return
        p, h = units[k]
        nA = 2 * p
        oy0 = HALF * h
        r0 = 2 * oy0 - 1           # input row of slab row 0
        ld0 = 1 if h == 0 else 0   # first valid slab row
        in0 = r0 + ld0             # first input row loaded

        xpv = xp[h][:].rearrange("p (r c) -> p r c", r=SROWS)
        spv = sp[h][:].rearrange("p (r c) -> p r c", r=SROWS)

        # k==0: band-split load+sign1 so the first conv starts early
        bands = ([(ld0, 15), (15, 29), (29, 43), (43, SROWS)] if k == 0
                 else [(ld0, SROWS)])
        na = min(ld0 + NA_ACT, SROWS)
        for (ra, rb) in bands:
            src = x_d[nA:nA + 2, :, r0 + ra:r0 + rb, :].rearrange(
                "i c r w -> (i c) r w")
            nc.sync.dma_start(xpv[:, ra:rb, :], src)
            if k == 0 or not sign1_bitwise:
                nc.scalar.activation(
                    spv[:, ra:rb, 1:113], xpv[:, ra:rb, :],
                    AF.Sign, bias=pt[:, PB11:PB11 + 1])
        if k > 0 and sign1_bitwise:
            # split ACT / DVE to balance engines.  DVE path: u32 bitwise
            # trick to fp32 +-1, then cast to fp8.
            nc.scalar.activation(
                spv[:, ld0:na, 1:113], xpv[:, ld0:na, :], AF.Sign)
            s1f = work1.tile([128, SROWS * W], f32, tag="s1f", name="s1f")
            flat = slice(na * W, SROWS * W)
            nc.vector.tensor_scalar(
                s1f[:, flat].bitcast(u32), xp[h][:, flat].bitcast(u32),
                0x80000000, 0x3F800000,
                ALU.bitwise_and, ALU.bitwise_or)
            s1v = s1f[:].rearrange("p (r c) -> p r c", r=SROWS)
            nc.vector.tensor_copy(spv[:, na:SROWS, 1:113],
                                  s1v[:, na:SROWS, :])

        # avgpool x4: row-pair add + col-pair add, exact fp32, on GPSIMD
        prow = work1.tile([128, HALF * W], f32, tag="prow", name="prow")
        prv = prow[:].rearrange("p (r c) -> p r c", r=HALF)
        nc.gpsimd.tensor_tensor(
            prv[:], xpv[:, 1:SROWS:2, :], xpv[:, 2:SROWS:2, :], ALU.add)
        s4 = work.tile([128, UN], f32, tag="s4", name="s4")
        s4v = s4[:].rearrange("p (r c) -> p r c", r=HALF)
        nc.gpsimd.tensor_tensor(
            s4v[:], prv[:, :, 0:W:2], prv[:, :, 1:W:2], ALU.add)
        s4s[k] = s4

    emit_a(0)
    for k, (p, h) in enumerate(units):
        nA, nB = 2 * p, 2 * p + 1
        oy0 = HALF * h
        s4 = s4s.pop(k)
        spv = sp[h][:].rearrange("p (r c) -> p r c", r=SROWS)

        # ---- conv1: 6 fp8 DoubleRow matmuls per chunk, both images via
        # block-diagonal weights into one [128, CN] psum tile ----
        u = work.tile([128, UN], f32, tag="u", name="u")
        for c in range(NCHUNK):
            cp = psum.tile([128, CN], f32, tag="cps", name="cps")
            for t in range(NDR):
                ky = t % 3
                kx0 = 0 if t < 3 else 2
                rs = ky + 14 * c
                rhs = spv[:, rs:rs + 13:2, kx0:kx0 + 112].rearrange(
                    "p r (ox two) -> p two r ox", two=2)
                nc.tensor.matmul(
                    cp[:], drw(t), rhs,
                    start=(t == 0), stop=(t == NDR - 1),
                    perf_mode=mybir.MatmulPerfMode.DoubleRow,
                )
            # u_c = 4*s3*conv + S4  (fused scalar_tensor_tensor on DVE)
            cs = slice(CN * c, CN * (c + 1))
            nc.vector.scalar_tensor_tensor(
                u[:, cs], cp[:], scal["s3x4"], s4[:, cs],
                ALU.mult, ALU.add)

        # hoist next unit's load/sign1/pool: its ACT/DVE/GPSIMD/DMA work
        # overlaps this unit's conv matmuls and stage-2
        emit_a(k + 1)

        # ---- prelu1 (-> bf16 out1) / sign2 ----
        out1 = work.tile([128, UN], bf16, tag="out1", name="out1")
        sg2 = work.tile([128, UN], bf16, tag="sg2", name="sg2")
        for c in range(NCHUNK):
            cs = slice(CN * c, CN * (c + 1))
            nc.scalar.activation(
                out1[:, cs], u[:, cs], AF.Prelu,
                bias=pt[:, PB12:PB12 + 1], scale=0.25,
                alpha=pt[:, PA1:PA1 + 1])
        if has_b13:
            nc.vector.tensor_scalar(
                out1[:], out1[:], pt[:, PB13:PB13 + 1], None, ALU.add)
        if fast_sign2:
            nc.scalar.activation(
                sg2[:], u[:], AF.Sign, bias=pt[:, PB12:PB12 + 1], scale=0.25)
        else:
            nc.scalar.activation(
                sg2[:], out1[:], AF.Sign, bias=pt[:, PBS2:PBS2 + 1])

        # ---- stage 2: per-image psum = (o1 | o2), residual injected ----
        # M=128 matmuls: lhsT [64, 128] = [wpw1|wpw2] then [diag1|diag2]
        stg = [work.tile([128, UN], bf16, tag=f"stg{i}", name=f"stg{i}")
               for i in range(2)]
        for i, n in enumerate((nA, nB)):
            pr = slice(64 * i, 64 * i + 64)   # rhs partitions (image i)
            for c in range(NCHUNK):
                cp = psum2.tile([128, CN], f32, tag=f"ps{i}", name=f"ps{i}")
                cs = slice(CN * c, CN * (c + 1))
                nc.tensor.matmul(
                    cp[:], w2[pr, O_PW:O_PW + 128], sg2[pr, cs],
                    start=True, stop=False)
                nc.tensor.matmul(
                    cp[:], w2[pr, O_DIAG:O_DIAG + 128], out1[pr, cs],
                    start=False, stop=True)
                nc.scalar.activation(
                    stg[i][:, cs], cp[:], AF.Prelu,
                    bias=pt[:, PB22F:PB22F + 1],
                    scale=pt[:, PS2V:PS2V + 1],
                    alpha=pt[:, PA2F:PA2F + 1])
            if has_b23:
                nc.vector.tensor_scalar(
                    stg[i][:], stg[i][:], pt[:, PB23F:PB23F + 1],
                    None, ALU.add)

        # ---- store (bf16): two 128-partition DMAs per image ----
        for i, n in enumerate((nA, nB)):
            sv = stg[i][:].rearrange("p (r c) -> p r c", r=HALF)
            hh = HALF // 2
            nc.sync.dma_start(y_d[n, :, oy0:oy0 + hh, :], sv[:, 0:hh, :])
            nc.sync.dma_start(y_d[n, :, oy0 + hh:oy0 + HALF, :],
                              sv[:, hh:HALF, :])

    for cm in reversed(pools):
        cm.__exit__(None, None, None)
    dram_cm.__exit__(None, None, None)
    tc_cm.__exit__(None, None, None)
    nc.compile()
    return nc, x_d.name, wdr_d.name, w2_d.name, p_d.name, y_d.name


def _prep(inputs):
    f32 = np.float32
    bf = ml_dtypes.bfloat16
    f8 = ml_dtypes.float8_e4m3fn
    w3 = np.asarray(inputs["w3"], f32)
    wpw1 = np.asarray(inputs["wpw1"], f32)
    wpw2 = np.asarray(inputs["wpw2"], f32)
    a1 = np.asarray(inputs["a1"], f32).reshape(CIN)
    a2 = np.asarray(inputs["a2"], f32).reshape(COUT)
    b11 = np.asarray(inputs["b11"], f32).reshape(CIN)
    b12 = np.asarray(inputs["b12"], f32).reshape(CIN)
    b13 = np.asarray(inputs["b13"], f32).reshape(CIN)
    b21 = np.asarray(inputs["b21"], f32).reshape(CIN)
    b22 = np.asarray(inputs["b22"], f32).reshape(COUT)
    b23 = np.asarray(inputs["b23"], f32).reshape(COUT)

    s3 = float(np.mean(np.abs(w3))) or 1.0
    s1 = float(np.mean(np.abs(wpw1))) or 1.0
    s2 = float(np.mean(np.abs(wpw2))) or 1.0

    # diag entries bf16(1/s_j); prelu2 scale 1/d_j compensates the rounding
    d1 = float(bf(1.0 / s1))
    d2 = float(bf(1.0 / s2))

    sgn = np.sign
    # fp8 DoubleRow conv weights: 6 blocks [128, 2, 128] block-diagonal over
    # the two images; plane order matches the rhs (ox two) factorization
    wdr = np.zeros((128, NDR, 2, 128), f32)
    for t in range(NDR):
        ky = t % 3
        kx0 = 0 if t < 3 else 2
        for i, kx in enumerate((kx0, kx0 + 1)):
            if kx > 2:
                continue  # zero plane
            wt = sgn(w3[:, :, ky, kx]).T       # [k=cin, m=cout]
            wdr[0:64, t, i, 0:64] = wt
            wdr[64:128, t, i, 64:128] = wt
    wdr8 = np.ascontiguousarray(
        wdr.reshape(128, NDR * DRW)).astype(f8)

    w2half = np.zeros((64, W2COLS), f32)
    w2half[:, O_PW:O_PW + 64] = sgn(wpw1[:, :, 0, 0]).T
    w2half[:, O_PW + 64:O_PW + 128] = sgn(wpw2[:, :, 0, 0]).T
    w2half[:, O_DIAG:O_DIAG + 64] = d1 * np.eye(64, dtype=f32)
    w2half[:, O_DIAG + 64:O_DIAG + 128] = d2 * np.eye(64, dtype=f32)
    w2full = np.concatenate([w2half, w2half], axis=0).astype(bf)

    def pairc(v):  # channel vec (64,) -> pair-layout (128,)
        return np.concatenate([v, v])

    params = np.zeros((128, NPARAM), f32)
    params[:, PA1] = pairc(a1)
    params[:, PB12] = pairc(b12)
    params[:, PB11] = pairc(b11)
    params[:, PA2F] = a2
    params[:, PB22F] = b22
    params[:, PS2V] = np.concatenate(
        [np.full(64, 1.0 / d1, f32), np.full(64, 1.0 / d2, f32)])
    params[:, PBS2] = pairc(b13 + b21)
    params[:, PB13] = pairc(b13)
    params[:, PB23F] = b23
    params[:, PS1] = 0.0

    scal = {
        "s3x4": 4.0 * s3,
        "fast_sign2": bool(np.all(b13 + b21 == 0.0) and np.all(a1 > 0)),
        "sign1_bitwise": bool(np.all(b11 == 0.0)),
        "has_b13": bool(np.any(b13 != 0.0)),
        "has_b23": bool(np.any(b23 != 0.0)),
    }
    return wdr8, w2full, params, scal


def kernel(**inputs):
    x = np.ascontiguousarray(np.asarray(inputs["x"], np.float32))
    wdr8, w2full, params, scal = _prep(inputs)

    key = tuple(sorted((k, v) for k, v in scal.items())) + (
        float(params.sum()),)
    if key not in _cache:
        _cache.clear()
        _cache[key] = _build(scal)
    nc, xn, wdrn, w2n, pn, yn = _cache[key]

    in_maps = []
    for i in range(NCORES):
        in_maps.append({
            xn: np.ascontiguousarray(x[BPC * i:BPC * (i + 1)]),
            wdrn: wdr8,
            w2n: w2full,
            pn: params,
        })
    res = bass_utils.run_bass_kernel_spmd(nc, in_maps,
                                          core_ids=list(range(NCORES)))
    out = np.concatenate(
        [res.results[i][yn].astype(np.float32) for i in range(NCORES)],
        axis=0)
    return out


# revision 11
# speedup vs baseline: 236.5208x; 1.2213x over previous
"""Trainium2 Bass kernel for the binarized BasicBlock (dense_cnn).

Contract: kernel(**inputs) takes the FULL unsharded inputs (numpy arrays,
keyed as in reference.setup_inputs()) and returns the FULL output
(32, 128, 56, 56) float32.  Internally shards the batch dim across 8
NeuronCores (pure data parallel, params replicated).

Per-core layout: 4 images as 2 pairs; each pair in 2 half-height units of
28 output rows.  Partitions hold (imgA ch0-63 | imgB ch0-63).  The binary
3x3 conv runs as fp8 DoubleRow matmuls: the stride-2 conv reads adjacent
sign-slab columns (2ox+kx, 2ox+kx+1), so taps (ky,0)+(ky,1) pair naturally
as the two DoubleRow K-planes and (ky,2) pairs with a zero plane -- 6 DR
matmuls per 7-row chunk compute both images into one PSUM tile via
block-diagonal weights.  The avgpool shortcut stays exact fp32 (DVE/GPSIMD
adds) so sign2 never flips vs the reference; sign1 is split between ACT
(AF.Sign -> fp8) and DVE (u32 bitwise trick + fp8 cast).  Stage-2 is the
proven bf16 path: [wpw1|wpw2] pw matmul + diag residual injection into
PSUM, PReLU with per-partition rounding-compensated scales.  Outputs are
stored bf16 and upcast on the host, halving store traffic.
"""
import sys

sys.path.insert(0, "/opt/trn_rl_repo")

import numpy as np
import ml_dtypes

import concourse.bacc as bacc
import concourse.mybir as mybir
import concourse.tile as tile
from concourse import bass_utils

# Problem shapes (hardcoded per spec)
B, CIN, H, W = 32, 64, 112, 112
COUT = 2 * CIN
NCORES = 8
BPC = B // NCORES          # images per core = 4
NPAIR = BPC // 2           # image pairs per core = 2
OH, OW = H // 2, W // 2    # 56, 56
HALF = OH // 2             # 28 output rows per unit
NCHUNK = 4                 # psum chunks per unit (7 out rows each)
CROWS = HALF // NCHUNK     # 7
CN = CROWS * OW            # 392 cols per chunk
UN = HALF * OW             # 1568 elems per unit (per partition)
SROWS = 57                 # raw/sign slab rows (input rows 2*oy0-1 .. 2*oy0+55)
SPITCH = 114               # sign slab col pitch (1 left pad + 112 + 1 right pad)
NA_ACT = 30                # sign1 slab rows on ACT; rest on DVE (balance)
GP_PROW = 28               # prow rows computed on GPSIMD; rest on DVE
NBANDS = 2                 # x-load DMA split per unit (steady state)

# param columns
PA1, PB12, PB11, PA2F, PB22F, PS2V, PBS2, PB13, PB23F, PS1 = range(10)
NPARAM = 10
# fp8 DoubleRow conv weight blocks: 6 blocks of [128, 2*128]
#   blocks 0..2: planes (ky,0),(ky,1) for ky=0,1,2
#   blocks 3..5: planes (ky,2),zero  for ky=0,1,2
NDR = 6
DRW = 2 * 128
# bf16 stage-2 weights: [wpw1|wpw2] then [diag1|diag2]
O_PW = 0
O_DIAG = 128
W2COLS = 256

_cache = {}


def _build(scal, reps=1):
    """Build the bass program. scal: host-derived scalars/flags.
    reps>1 replicates the whole compute (for slope-based device timing)."""
    nc = bacc.Bacc("TRN2", target_bir_lowering=False, debug=False)
    f32 = mybir.dt.float32
    bf16 = mybir.dt.bfloat16
    fp8 = mybir.dt.float8e4
    u32 = mybir.dt.uint32
    AF = mybir.ActivationFunctionType
    ALU = mybir.AluOpType

    fast_sign2 = scal["fast_sign2"]
    sign1_bitwise = scal["sign1_bitwise"]
    has_b13 = scal["has_b13"]
    has_b23 = scal["has_b23"]

    tc_cm = tile.TileContext(nc)
    tc = tc_cm.__enter__()
    dram_cm = tc.tile_pool(name="dram", bufs=1, space="DRAM")
    dram = dram_cm.__enter__()

    x_d = dram.tile([BPC, CIN, H, W], f32, kind="ExternalInput")
    wdr_d = dram.tile([128, NDR * DRW], fp8, kind="ExternalInput")
    w2_d = dram.tile([128, W2COLS], bf16, kind="ExternalInput")
    p_d = dram.tile([128, NPARAM], f32, kind="ExternalInput")
    y_d = dram.tile([BPC, COUT, OH, OW], bf16, kind="ExternalOutput")

    pools = []

    def pool(name, **kw):
        cm = tc.tile_pool(name=name, **kw)
        pools.append(cm)
        return cm.__enter__()

    const = pool("const", bufs=1)
    pers = pool("pers", bufs=1)
    work = pool("work", bufs=2)
    work1 = pool("work1", bufs=1)
    psum = pool("psum", bufs=4, space="PSUM")
    psum2 = pool("psum2", bufs=2, space="PSUM")

    wdr = const.tile([128, NDR * DRW], fp8)
    w2 = const.tile([128, W2COLS], bf16)
    pt = const.tile([128, NPARAM], f32)
    nc.sync.dma_start(wdr[:], wdr_d[:])
    nc.sync.dma_start(w2[:], w2_d[:])
    nc.sync.dma_start(pt[:], p_d[:])

    # persistent slabs: index by half h (stable pad semantics per buffer)
    xp = [pers.tile([128, SROWS * W], f32, tag=f"xp{h}", name=f"xp{h}")
          for h in range(2)]
    sp = [pers.tile([128, SROWS * SPITCH], fp8, tag=f"sp{h}", name=f"sp{h}")
          for h in range(2)]
    for h in range(2):
        # zero only the pad borders (row 0, col 0, cols 112..113 are data or
        # right pad; col 113 is the DoubleRow zero-plane read for ox=55)
        spv0 = sp[h][:].rearrange("p (r c) -> p r c", r=SROWS)
        nc.vector.memset(spv0[:, 0:1, :], 0.0)
        nc.vector.memset(spv0[:, :, 0:1], 0.0)
        nc.vector.memset(spv0[:, :, 113:114], 0.0)

    def drw(blk):
        # DoubleRow lhsT view for block blk: [128, 2, 128]
        return wdr[:, DRW * blk:DRW * blk + DRW].rearrange(
            "p (two m) -> p two m", two=2)

    units = [(p, h) for _ in range(reps)
             for p in range(NPAIR) for h in range(2)]
    s4s = {}

    def emit_a(k):
        """Phase A of unit k: x load, sign1 -> sp, avgpool -> s4."""
        if k >= len(units):
            return
        p, h = units[k]
        nA = 2 * p
        oy0 = HALF * h
        r0 = 2 * oy0 - 1           # input row of slab row 0
        ld0 = 1 if h == 0 else 0   # first valid slab row
        in0 = r0 + ld0             # first input row loaded

        xpv = xp[h][:].rearrange("p (r c) -> p r c", r=SROWS)
        spv = sp[h][:].rearrange("p (r c) -> p r c", r=SROWS)

        # 4 chunk-aligned bands: DMA -> sign1 (ACT/DVE split) -> prow
        # quarter (GPSIMD) -> s4 quarter (GPSIMD), so s4 and the sign slab
        # become ready chunk-by-chunk and the next unit's chunk 0 can start
        # as soon as its first band lands.
        bands = [(ld0, 15), (15, 29), (29, 43), (43, SROWS)]
        na = min(ld0 + NA_ACT, SROWS)
        prow = work1.tile([128, HALF * W], f32, tag="prow", name="prow")
        prv = prow[:].rearrange("p (r c) -> p r c", r=HALF)
        s1f = work1.tile([128, SROWS * W], f32, tag="s1f", name="s1f")
        s1v = s1f[:].rearrange("p (r c) -> p r c", r=SROWS)
        s4 = work.tile([128, UN], f32, tag="s4", name="s4")
        s4v = s4[:].rearrange("p (r c) -> p r c", r=HALF)
        for b, (ra, rb) in enumerate(bands):
            src = x_d[nA:nA + 2, :, r0 + ra:r0 + rb, :].rearrange(
                "i c r w -> (i c) r w")
            nc.sync.dma_start(xpv[:, ra:rb, :], src)
            # sign1 for this band
            if sign1_bitwise and k > 0:
                aa, ab = ra, min(rb, na)       # ACT rows
                da, db = max(ra, na), rb       # DVE rows
                if ab > aa:
                    nc.scalar.activation(
                        spv[:, aa:ab, 1:113], xpv[:, aa:ab, :], AF.Sign)
                if db > da:
                    flat = slice(da * W, db * W)
                    nc.vector.tensor_scalar(
                        s1f[:, flat].bitcast(u32),
                        xp[h][:, flat].bitcast(u32),
                        0x80000000, 0x3F800000,
                        ALU.bitwise_and, ALU.bitwise_or)
                    nc.vector.tensor_copy(spv[:, da:db, 1:113],
                                          s1v[:, da:db, :])
            else:
                nc.scalar.activation(
                    spv[:, ra:rb, 1:113], xpv[:, ra:rb, :],
                    AF.Sign, bias=pt[:, PB11:PB11 + 1])
            # avgpool quarter: prow rows [7b, 7b+7) need xp rows
            # [14b+1, 14b+15) which this band covers
            p0, p1 = 7 * b, 7 * b + 7
            nc.gpsimd.tensor_tensor(
                prv[:, p0:p1, :], xpv[:, 2 * p0 + 1:2 * p1:2, :],
                xpv[:, 2 * p0 + 2:2 * p1 + 1:2, :], ALU.add)
            nc.gpsimd.tensor_tensor(
                s4v[:, p0:p1, :], prv[:, p0:p1, 0:W:2],
                prv[:, p0:p1, 1:W:2], ALU.add)
        s4s[k] = s4

    emit_a(0)
    for k, (p, h) in enumerate(units):
        nA, nB = 2 * p, 2 * p + 1
        oy0 = HALF * h
        s4 = s4s.pop(k)
        spv = sp[h][:].rearrange("p (r c) -> p r c", r=SROWS)
        # prefetch the next unit first: its DMA/sign1/pool fill the other
        # slab while this unit computes
        emit_a(k + 1)

        # ---- fully chunk-pipelined main body: conv_c -> stt_c -> prelu1_c
        # -> sign2_c -> stage2_c -> prelu2_c, so consecutive chunks overlap
        # across PE/DVE/ACT and the PE stream stays dense ----
        u = work.tile([128, UN], f32, tag="u", name="u")
        out1 = work.tile([128, UN], bf16, tag="out1", name="out1")
        sg2 = work.tile([128, UN], bf16, tag="sg2", name="sg2")
        stg = [work.tile([128, UN], bf16, tag=f"stg{i}", name=f"stg{i}")
               for i in range(2)]
        fused = fast_sign2 and not has_b13
        for c in range(NCHUNK):
            cp = psum.tile([128, CN], f32, tag="cps", name="cps")
            for t in range(NDR):
                ky = t % 3
                kx0 = 0 if t < 3 else 2
                rs = ky + 14 * c
                rhs = spv[:, rs:rs + 13:2, kx0:kx0 + 112].rearrange(
                    "p r (ox two) -> p two r ox", two=2)
                nc.tensor.matmul(
                    cp[:], drw(t), rhs,
                    start=(t == 0), stop=(t == NDR - 1),
                    perf_mode=mybir.MatmulPerfMode.DoubleRow,
                )
            # u_c = 4*s3*conv + S4  (fused scalar_tensor_tensor on DVE)
            cs = slice(CN * c, CN * (c + 1))
            nc.vector.scalar_tensor_tensor(
                u[:, cs], cp[:], scal["s3x4"], s4[:, cs],
                ALU.mult, ALU.add)
            nc.scalar.activation(
                out1[:, cs], u[:, cs], AF.Prelu,
                bias=pt[:, PB12:PB12 + 1], scale=0.25,
                alpha=pt[:, PA1:PA1 + 1])
            if fused:
                nc.scalar.activation(
                    sg2[:, cs], u[:, cs], AF.Sign,
                    bias=pt[:, PB12:PB12 + 1], scale=0.25)
                for i in range(2):
                    pr = slice(64 * i, 64 * i + 64)
                    cp2 = psum2.tile([128, CN], f32, tag=f"ps{i}",
                                     name=f"ps{i}")
                    nc.tensor.matmul(
                        cp2[:], w2[pr, O_PW:O_PW + 128], sg2[pr, cs],
                        start=True, stop=False)
                    nc.tensor.matmul(
                        cp2[:], w2[pr, O_DIAG:O_DIAG + 128], out1[pr, cs],
                        start=False, stop=True)
                    nc.scalar.activation(
                        stg[i][:, cs], cp2[:], AF.Prelu,
                        bias=pt[:, PB22F:PB22F + 1],
                        scale=pt[:, PS2V:PS2V + 1],
                        alpha=pt[:, PA2F:PA2F + 1])


        if not fused:
            # general fallback (nonzero b13/b21 or non-positive alpha):
            # unit-level sign2/stage2 as in the baseline kernel
            if has_b13:
                nc.vector.tensor_scalar(
                    out1[:], out1[:], pt[:, PB13:PB13 + 1], None, ALU.add)
            if fast_sign2:
                nc.scalar.activation(
                    sg2[:], u[:], AF.Sign,
                    bias=pt[:, PB12:PB12 + 1], scale=0.25)
            else:
                nc.scalar.activation(
                    sg2[:], out1[:], AF.Sign, bias=pt[:, PBS2:PBS2 + 1])
            for i, n in enumerate((nA, nB)):
                pr = slice(64 * i, 64 * i + 64)
                for c in range(NCHUNK):
                    cp2 = psum2.tile([128, CN], f32, tag=f"ps{i}",
                                     name=f"ps{i}")
                    cs = slice(CN * c, CN * (c + 1))
                    nc.tensor.matmul(
                        cp2[:], w2[pr, O_PW:O_PW + 128], sg2[pr, cs],
                        start=True, stop=False)
                    nc.tensor.matmul(
                        cp2[:], w2[pr, O_DIAG:O_DIAG + 128], out1[pr, cs],
                        start=False, stop=True)
                    nc.scalar.activation(
                        stg[i][:, cs], cp2[:], AF.Prelu,
                        bias=pt[:, PB22F:PB22F + 1],
                        scale=pt[:, PS2V:PS2V + 1],
                        alpha=pt[:, PA2F:PA2F + 1])
                if has_b23:
                    nc.vector.tensor_scalar(
                        stg[i][:], stg[i][:], pt[:, PB23F:PB23F + 1],
                        None, ALU.add)

        # ---- store (bf16): two 128-partition DMAs per image ----
        for i, n in enumerate((nA, nB)):
            sv = stg[i][:].rearrange("p (r c) -> p r c", r=HALF)
            hh = HALF // 2
            nc.sync.dma_start(y_d[n, :, oy0:oy0 + hh, :], sv[:, 0:hh, :])
            nc.sync.dma_start(y_d[n, :, oy0 + hh:oy0 + HALF, :],
                              sv[:, hh:HALF, :])

    for cm in reversed(pools):
        cm.__exit__(None, None, None)
    dram_cm.__exit__(None, None, None)
    tc_cm.__exit__(None, None, None)
    nc.compile()
    return nc, x_d.name, wdr_d.name, w2_d.name, p_d.name, y_d.name


def _prep(inputs):
    f32 = np.float32
    bf = ml_dtypes.bfloat16
    f8 = ml_dtypes.float8_e4m3fn
    w3 = np.asarray(inputs["w3"], f32)
    wpw1 = np.asarray(inputs["wpw1"], f32)
    wpw2 = np.asarray(inputs["wpw2"], f32)
    a1 = np.asarray(inputs["a1"], f32).reshape(CIN)
    a2 = np.asarray(inputs["a2"], f32).reshape(COUT)
    b11 = np.asarray(inputs["b11"], f32).reshape(CIN)
    b12 = np.asarray(inputs["b12"], f32).reshape(CIN)
    b13 = np.asarray(inputs["b13"], f32).reshape(CIN)
    b21 = np.asarray(inputs["b21"], f32).reshape(CIN)
    b22 = np.asarray(inputs["b22"], f32).reshape(COUT)
    b23 = np.asarray(inputs["b23"], f32).reshape(COUT)

    s3 = float(np.mean(np.abs(w3))) or 1.0
    s1 = float(np.mean(np.abs(wpw1))) or 1.0
    s2 = float(np.mean(np.abs(wpw2))) or 1.0

    # diag entries bf16(1/s_j); prelu2 scale 1/d_j compensates the rounding
    d1 = float(bf(1.0 / s1))
    d2 = float(bf(1.0 / s2))

    sgn = np.sign
    # fp8 DoubleRow conv weights: 6 blocks [128, 2, 128] block-diagonal over
    # the two images; plane order matches the rhs (ox two) factorization
    wdr = np.zeros((128, NDR, 2, 128), f32)
    for t in range(NDR):
        ky = t % 3
        kx0 = 0 if t < 3 else 2
        for i, kx in enumerate((kx0, kx0 + 1)):
            if kx > 2:
                continue  # zero plane
            wt = sgn(w3[:, :, ky, kx]).T       # [k=cin, m=cout]
            wdr[0:64, t, i, 0:64] = wt
            wdr[64:128, t, i, 64:128] = wt
    wdr8 = np.ascontiguousarray(
        wdr.reshape(128, NDR * DRW)).astype(f8)

    w2half = np.zeros((64, W2COLS), f32)
    w2half[:, O_PW:O_PW + 64] = sgn(wpw1[:, :, 0, 0]).T
    w2half[:, O_PW + 64:O_PW + 128] = sgn(wpw2[:, :, 0, 0]).T
    w2half[:, O_DIAG:O_DIAG + 64] = d1 * np.eye(64, dtype=f32)
    w2half[:, O_DIAG + 64:O_DIAG + 128] = d2 * np.eye(64, dtype=f32)
    w2full = np.concatenate([w2half, w2half], axis=0).astype(bf)

    def pairc(v):  # channel vec (64,) -> pair-layout (128,)
        return np.concatenate([v, v])

    params = np.zeros((128, NPARAM), f32)
    params[:, PA1] = pairc(a1)
    params[:, PB12] = pairc(b12)
    params[:, PB11] = pairc(b11)
    params[:, PA2F] = a2
    params[:, PB22F] = b22
    params[:, PS2V] = np.concatenate(
        [np.full(64, 1.0 / d1, f32), np.full(64, 1.0 / d2, f32)])
    params[:, PBS2] = pairc(b13 + b21)
    params[:, PB13] = pairc(b13)
    params[:, PB23F] = b23
    params[:, PS1] = 0.0

    scal = {
        "s3x4": 4.0 * s3,
        "fast_sign2": bool(np.all(b13 + b21 == 0.0) and np.all(a1 > 0)),
        "sign1_bitwise": bool(np.all(b11 == 0.0)),
        "has_b13": bool(np.any(b13 != 0.0)),
        "has_b23": bool(np.any(b23 != 0.0)),
    }
    return wdr8, w2full, params, scal


def kernel(**inputs):
    x = np.ascontiguousarray(np.asarray(inputs["x"], np.float32))
    wdr8, w2full, params, scal = _prep(inputs)

    key = tuple(sorted((k, v) for k, v in scal.items())) + (
        float(params.sum()),)
    if key not in _cache:
        _cache.clear()
        _cache[key] = _build(scal)
    nc, xn, wdrn, w2n, pn, yn = _cache[key]

    in_maps = []
    for i in range(NCORES):
        in_maps.append({
            xn: np.ascontiguousarray(x[BPC * i:BPC * (i + 1)]),
            wdrn: wdr8,
            w2n: w2full,
            pn: params,
        })
    res = bass_utils.run_bass_kernel_spmd(nc, in_maps,
                                          core_ids=list(range(NCORES)))
    out = np.concatenate(
        [res.results[i][yn].astype(np.float32) for i in range(NCORES)],
        axis=0)
    return out


# revision 18
# speedup vs baseline: 238.5394x; 1.0085x over previous
"""Trainium2 Bass kernel for the binarized BasicBlock (dense_cnn).

Contract: kernel(**inputs) takes the FULL unsharded inputs (numpy arrays,
keyed as in reference.setup_inputs()) and returns the FULL output
(32, 128, 56, 56) float32.  Internally shards the batch dim across 8
NeuronCores (pure data parallel, params replicated).

Per-core layout: 4 images as 2 pairs; each pair in 2 half-height units of
28 output rows.  Partitions hold (imgA ch0-63 | imgB ch0-63).  The binary
3x3 conv runs as fp8 DoubleRow matmuls: the stride-2 conv reads adjacent
sign-slab columns (2ox+kx, 2ox+kx+1), so taps (ky,0)+(ky,1) pair naturally
as the two DoubleRow K-planes and (ky,2) pairs with a zero plane -- 6 DR
matmuls per 7-row chunk compute both images into one PSUM tile via
block-diagonal weights.  The avgpool shortcut stays exact fp32 (DVE/GPSIMD
adds) so sign2 never flips vs the reference; sign1 is split between ACT
(AF.Sign -> fp8) and DVE (u32 bitwise trick + fp8 cast).  Stage-2 is the
proven bf16 path: [wpw1|wpw2] pw matmul + diag residual injection into
PSUM, PReLU with per-partition rounding-compensated scales.  Outputs are
stored bf16 and upcast on the host, halving store traffic.
"""
import sys

sys.path.insert(0, "/opt/trn_rl_repo")

import numpy as np
import ml_dtypes

import concourse.bacc as bacc
import concourse.mybir as mybir
import concourse.tile as tile
from concourse import bass_utils

# Problem shapes (hardcoded per spec)
B, CIN, H, W = 32, 64, 112, 112
COUT = 2 * CIN
NCORES = 8
BPC = B // NCORES          # images per core = 4
NPAIR = BPC // 2           # image pairs per core = 2
OH, OW = H // 2, W // 2    # 56, 56
HALF = OH // 2             # 28 output rows per unit
NCHUNK = 4                 # psum chunks per unit (7 out rows each)
CROWS = HALF // NCHUNK     # 7
CN = CROWS * OW            # 392 cols per chunk
UN = HALF * OW             # 1568 elems per unit (per partition)
SROWS = 57                 # raw/sign slab rows (input rows 2*oy0-1 .. 2*oy0+55)
SPITCH = 114               # sign slab col pitch (1 left pad + 112 + 1 right pad)
NA_ACT = 38                # sign1 slab rows on ACT; rest on DVE (balance)
GP_PROW = 28               # prow rows computed on GPSIMD; rest on DVE
NBANDS = 2                 # x-load DMA split per unit (steady state)

# param columns
PA1, PB12, PB11, PA2F, PB22F, PS2V, PBS2, PB13, PB23F, PS1 = range(10)
NPARAM = 10
# fp8 DoubleRow conv weight blocks: 6 blocks of [128, 2*128]
#   blocks 0..2: planes (ky,0),(ky,1) for ky=0,1,2
#   blocks 3..5: planes (ky,2),zero  for ky=0,1,2
NDR = 6
DRW = 2 * 128
# bf16 stage-2 weights: [wpw1|wpw2] then [diag1|diag2]
O_PW = 0
O_DIAG = 128
W2COLS = 256

_cache = {}


def _build(scal, reps=1):
    """Build the bass program. scal: host-derived scalars/flags.
    reps>1 replicates the whole compute (for slope-based device timing)."""
    nc = bacc.Bacc("TRN2", target_bir_lowering=False, debug=False)
    f32 = mybir.dt.float32
    bf16 = mybir.dt.bfloat16
    fp8 = mybir.dt.float8e4
    u32 = mybir.dt.uint32
    AF = mybir.ActivationFunctionType
    ALU = mybir.AluOpType

    fast_sign2 = scal["fast_sign2"]
    sign1_bitwise = scal["sign1_bitwise"]
    sign2_bitwise = scal["sign2_bitwise"]
    has_b13 = scal["has_b13"]
    has_b23 = scal["has_b23"]

    tc_cm = tile.TileContext(nc)
    tc = tc_cm.__enter__()
    dram_cm = tc.tile_pool(name="dram", bufs=1, space="DRAM")
    dram = dram_cm.__enter__()

    x_d = dram.tile([BPC, CIN, H, W], f32, kind="ExternalInput")
    wdr_d = dram.tile([128, NDR * DRW], fp8, kind="ExternalInput")
    w2_d = dram.tile([128, W2COLS], bf16, kind="ExternalInput")
    p_d = dram.tile([128, NPARAM], f32, kind="ExternalInput")
    y_d = dram.tile([BPC, COUT, OH, OW], bf16, kind="ExternalOutput")

    pools = []

    def pool(name, **kw):
        cm = tc.tile_pool(name=name, **kw)
        pools.append(cm)
        return cm.__enter__()

    const = pool("const", bufs=1)
    pers = pool("pers", bufs=1)
    work = pool("work", bufs=2)
    work1 = pool("work1", bufs=1)
    psum = pool("psum", bufs=4, space="PSUM")
    psum2 = pool("psum2", bufs=2, space="PSUM")

    wdr = const.tile([128, NDR * DRW], fp8)
    w2 = const.tile([128, W2COLS], bf16)
    pt = const.tile([128, NPARAM], f32)
    nc.sync.dma_start(wdr[:], wdr_d[:])
    nc.sync.dma_start(w2[:], w2_d[:])
    nc.sync.dma_start(pt[:], p_d[:])

    # persistent slabs: index by half h (stable pad semantics per buffer)
    xp = [pers.tile([128, SROWS * W], f32, tag=f"xp{h}", name=f"xp{h}")
          for h in range(2)]
    sp = [pers.tile([128, SROWS * SPITCH], fp8, tag=f"sp{h}", name=f"sp{h}")
          for h in range(2)]
    for h in range(2):
        # zero only the pad borders (row 0, col 0, cols 112..113 are data or
        # right pad; col 113 is the DoubleRow zero-plane read for ox=55)
        spv0 = sp[h][:].rearrange("p (r c) -> p r c", r=SROWS)
        nc.vector.memset(spv0[:, 0:1, :], 0.0)
        nc.vector.memset(spv0[:, :, 0:1], 0.0)
        nc.vector.memset(spv0[:, :, 113:114], 0.0)

    def drw(blk):
        # DoubleRow lhsT view for block blk: [128, 2, 128]
        return wdr[:, DRW * blk:DRW * blk + DRW].rearrange(
            "p (two m) -> p two m", two=2)

    units = [(p, h) for _ in range(reps)
             for p in range(NPAIR) for h in range(2)]
    s4s = {}

    BANDS = [(0, 15), (15, 29), (29, 43), (43, SROWS)]

    def _geom(k):
        p, h = units[k]
        r0 = 2 * HALF * h - 1      # input row of slab row 0
        ld0 = 1 if h == 0 else 0   # first valid slab row
        return 2 * p, h, r0, ld0

    def emit_dma(k):
        """x band loads for unit k (2 units ahead of compute)."""
        if k >= len(units):
            return
        nA, h, r0, ld0 = _geom(k)
        xpv = xp[h][:].rearrange("p (r c) -> p r c", r=SROWS)
        for (ra, rb) in BANDS:
            ra = max(ra, ld0)
            src = x_d[nA:nA + 2, :, r0 + ra:r0 + rb, :].rearrange(
                "i c r w -> (i c) r w")
            nc.sync.dma_start(xpv[:, ra:rb, :], src)

    def emit_a(k):
        """Phase A of unit k: sign1 -> sp, avgpool -> s4, per band."""
        if k >= len(units):
            return
        nA, h, r0, ld0 = _geom(k)
        xpv = xp[h][:].rearrange("p (r c) -> p r c", r=SROWS)
        spv = sp[h][:].rearrange("p (r c) -> p r c", r=SROWS)
        na = min(ld0 + NA_ACT, SROWS)
        prow = work1.tile([128, HALF * W], f32, tag="prow", name="prow")
        prv = prow[:].rearrange("p (r c) -> p r c", r=HALF)
        s1f = work1.tile([128, SROWS * W], f32, tag="s1f", name="s1f")
        s1v = s1f[:].rearrange("p (r c) -> p r c", r=SROWS)
        s4 = work.tile([128, UN], f32, tag="s4", name="s4")
        s4v = s4[:].rearrange("p (r c) -> p r c", r=HALF)
        for b, (ra, rb) in enumerate(BANDS):
            ra = max(ra, ld0)
            # sign1 for this band
            if sign1_bitwise and k > 0:
                aa, ab = ra, min(rb, na)       # ACT rows
                da, db = max(ra, na), rb       # DVE rows
                if ab > aa:
                    nc.scalar.activation(
                        spv[:, aa:ab, 1:113], xpv[:, aa:ab, :], AF.Sign)
                if db > da:
                    flat = slice(da * W, db * W)
                    nc.vector.tensor_scalar(
                        s1f[:, flat].bitcast(u32),
                        xp[h][:, flat].bitcast(u32),
                        0x80000000, 0x3F800000,
                        ALU.bitwise_and, ALU.bitwise_or)
                    nc.vector.tensor_copy(spv[:, da:db, 1:113],
                                          s1v[:, da:db, :])
            else:
                nc.scalar.activation(
                    spv[:, ra:rb, 1:113], xpv[:, ra:rb, :],
                    AF.Sign, bias=pt[:, PB11:PB11 + 1])
            # avgpool quarter: prow rows [7b, 7b+7) need xp rows
            # [14b+1, 14b+15) which this band covers
            p0, p1 = 7 * b, 7 * b + 7
            nc.gpsimd.tensor_tensor(
                prv[:, p0:p1, :], xpv[:, 2 * p0 + 1:2 * p1:2, :],
                xpv[:, 2 * p0 + 2:2 * p1 + 1:2, :], ALU.add)
            nc.gpsimd.tensor_tensor(
                s4v[:, p0:p1, :], prv[:, p0:p1, 0:W:2],
                prv[:, p0:p1, 1:W:2], ALU.add)
        s4s[k] = s4

    emit_dma(0)
    emit_dma(1)
    emit_a(0)
    for k, (p, h) in enumerate(units):
        nA, nB = 2 * p, 2 * p + 1
        oy0 = HALF * h
        s4 = s4s.pop(k)
        spv = sp[h][:].rearrange("p (r c) -> p r c", r=SROWS)
        # 2-deep DMA prefetch (xp[h] readers finish early; only sp[h] is
        # read until unit end), 1-deep for sign1/pool of the next unit
        emit_dma(k + 2)
        emit_a(k + 1)

        # ---- fully chunk-pipelined main body: conv_c -> stt_c -> prelu1_c
        # -> sign2_c -> stage2_c -> prelu2_c, so consecutive chunks overlap
        # across PE/DVE/ACT and the PE stream stays dense ----
        u = work.tile([128, UN], f32, tag="u", name="u")
        out1 = work.tile([128, UN], bf16, tag="out1", name="out1")
        sg2 = work.tile([128, UN], bf16, tag="sg2", name="sg2")
        sg2f = work.tile([128, UN], f32, tag="sg2f", name="sg2f")
        stg = [work.tile([128, UN], bf16, tag=f"stg{i}", name=f"stg{i}")
               for i in range(2)]
        fused = fast_sign2 and not has_b13
        for c in range(NCHUNK):
            cp = psum.tile([128, CN], f32, tag="cps", name="cps")
            for t in range(NDR):
                ky = t % 3
                kx0 = 0 if t < 3 else 2
                rs = ky + 14 * c
                rhs = spv[:, rs:rs + 13:2, kx0:kx0 + 112].rearrange(
                    "p r (ox two) -> p two r ox", two=2)
                nc.tensor.matmul(
                    cp[:], drw(t), rhs,
                    start=(t == 0), stop=(t == NDR - 1),
                    perf_mode=mybir.MatmulPerfMode.DoubleRow,
                )
            # u_c = 4*s3*conv + S4  (fused scalar_tensor_tensor on DVE)
            cs = slice(CN * c, CN * (c + 1))
            nc.vector.scalar_tensor_tensor(
                u[:, cs], cp[:], scal["s3x4"], s4[:, cs],
                ALU.mult, ALU.add)
            nc.scalar.activation(
                out1[:, cs], u[:, cs], AF.Prelu,
                bias=pt[:, PB12:PB12 + 1], scale=0.25,
                alpha=pt[:, PA1:PA1 + 1])
            if fused:
                if sign2_bitwise:
                    # sign(0.25*u + b12) with b12==0: u32 bitwise trick on
                    # DVE (ACT is the bottleneck engine), then cast to bf16
                    nc.vector.tensor_scalar(
                        sg2f[:, cs].bitcast(u32), u[:, cs].bitcast(u32),
                        0x80000000, 0x3F800000,
                        ALU.bitwise_and, ALU.bitwise_or)
                    nc.vector.tensor_copy(sg2[:, cs], sg2f[:, cs])
                else:
                    nc.scalar.activation(
                        sg2[:, cs], u[:, cs], AF.Sign,
                        bias=pt[:, PB12:PB12 + 1], scale=0.25)
                for i in range(2):
                    pr = slice(64 * i, 64 * i + 64)
                    cp2 = psum2.tile([128, CN], f32, tag=f"ps{i}",
                                     name=f"ps{i}")
                    nc.tensor.matmul(
                        cp2[:], w2[pr, O_PW:O_PW + 128], sg2[pr, cs],
                        start=True, stop=False)
                    nc.tensor.matmul(
                        cp2[:], w2[pr, O_DIAG:O_DIAG + 128], out1[pr, cs],
                        start=False, stop=True)
                    nc.scalar.activation(
                        stg[i][:, cs], cp2[:], AF.Prelu,
                        bias=pt[:, PB22F:PB22F + 1],
                        scale=pt[:, PS2V:PS2V + 1],
                        alpha=pt[:, PA2F:PA2F + 1])


        if not fused:
            # general fallback (nonzero b13/b21 or non-positive alpha):
            # unit-level sign2/stage2 as in the baseline kernel
            if has_b13:
                nc.vector.tensor_scalar(
                    out1[:], out1[:], pt[:, PB13:PB13 + 1], None, ALU.add)
            if fast_sign2:
                nc.scalar.activation(
                    sg2[:], u[:], AF.Sign,
                    bias=pt[:, PB12:PB12 + 1], scale=0.25)
            else:
                nc.scalar.activation(
                    sg2[:], out1[:], AF.Sign, bias=pt[:, PBS2:PBS2 + 1])
            for i, n in enumerate((nA, nB)):
                pr = slice(64 * i, 64 * i + 64)
                for c in range(NCHUNK):
                    cp2 = psum2.tile([128, CN], f32, tag=f"ps{i}",
                                     name=f"ps{i}")
                    cs = slice(CN * c, CN * (c + 1))
                    nc.tensor.matmul(
                        cp2[:], w2[pr, O_PW:O_PW + 128], sg2[pr, cs],
                        start=True, stop=False)
                    nc.tensor.matmul(
                        cp2[:], w2[pr, O_DIAG:O_DIAG + 128], out1[pr, cs],
                        start=False, stop=True)
                    nc.scalar.activation(
                        stg[i][:, cs], cp2[:], AF.Prelu,
                        bias=pt[:, PB22F:PB22F + 1],
                        scale=pt[:, PS2V:PS2V + 1],
                        alpha=pt[:, PA2F:PA2F + 1])
                if has_b23:
                    nc.vector.tensor_scalar(
                        stg[i][:], stg[i][:], pt[:, PB23F:PB23F + 1],
                        None, ALU.add)

        # ---- store (bf16): two 128-partition DMAs per image ----
        for i, n in enumerate((nA, nB)):
            sv = stg[i][:].rearrange("p (r c) -> p r c", r=HALF)
            hh = HALF // 2
            nc.sync.dma_start(y_d[n, :, oy0:oy0 + hh, :], sv[:, 0:hh, :])
            nc.sync.dma_start(y_d[n, :, oy0 + hh:oy0 + HALF, :],
                              sv[:, hh:HALF, :])

    for cm in reversed(pools):
        cm.__exit__(None, None, None)
    dram_cm.__exit__(None, None, None)
    tc_cm.__exit__(None, None, None)
    nc.compile()
    return nc, x_d.name, wdr_d.name, w2_d.name, p_d.name, y_d.name


def _prep(inputs):
    f32 = np.float32
    bf = ml_dtypes.bfloat16
    f8 = ml_dtypes.float8_e4m3fn
    w3 = np.asarray(inputs["w3"], f32)
    wpw1 = np.asarray(inputs["wpw1"], f32)
    wpw2 = np.asarray(inputs["wpw2"], f32)
    a1 = np.asarray(inputs["a1"], f32).reshape(CIN)
    a2 = np.asarray(inputs["a2"], f32).reshape(COUT)
    b11 = np.asarray(inputs["b11"], f32).reshape(CIN)
    b12 = np.asarray(inputs["b12"], f32).reshape(CIN)
    b13 = np.asarray(inputs["b13"], f32).reshape(CIN)
    b21 = np.asarray(inputs["b21"], f32).reshape(CIN)
    b22 = np.asarray(inputs["b22"], f32).reshape(COUT)
    b23 = np.asarray(inputs["b23"], f32).reshape(COUT)

    s3 = float(np.mean(np.abs(w3))) or 1.0
    s1 = float(np.mean(np.abs(wpw1))) or 1.0
    s2 = float(np.mean(np.abs(wpw2))) or 1.0

    # diag entries bf16(1/s_j); prelu2 scale 1/d_j compensates the rounding
    d1 = float(bf(1.0 / s1))
    d2 = float(bf(1.0 / s2))

    sgn = np.sign
    # fp8 DoubleRow conv weights: 6 blocks [128, 2, 128] block-diagonal over
    # the two images; plane order matches the rhs (ox two) factorization
    wdr = np.zeros((128, NDR, 2, 128), f32)
    for t in range(NDR):
        ky = t % 3
        kx0 = 0 if t < 3 else 2
        for i, kx in enumerate((kx0, kx0 + 1)):
            if kx > 2:
                continue  # zero plane
            wt = sgn(w3[:, :, ky, kx]).T       # [k=cin, m=cout]
            wdr[0:64, t, i, 0:64] = wt
            wdr[64:128, t, i, 64:128] = wt
    wdr8 = np.ascontiguousarray(
        wdr.reshape(128, NDR * DRW)).astype(f8)

    w2half = np.zeros((64, W2COLS), f32)
    w2half[:, O_PW:O_PW + 64] = sgn(wpw1[:, :, 0, 0]).T
    w2half[:, O_PW + 64:O_PW + 128] = sgn(wpw2[:, :, 0, 0]).T
    w2half[:, O_DIAG:O_DIAG + 64] = d1 * np.eye(64, dtype=f32)
    w2half[:, O_DIAG + 64:O_DIAG + 128] = d2 * np.eye(64, dtype=f32)
    w2full = np.concatenate([w2half, w2half], axis=0).astype(bf)

    def pairc(v):  # channel vec (64,) -> pair-layout (128,)
        return np.concatenate([v, v])

    params = np.zeros((128, NPARAM), f32)
    params[:, PA1] = pairc(a1)
    params[:, PB12] = pairc(b12)
    params[:, PB11] = pairc(b11)
    params[:, PA2F] = a2
    params[:, PB22F] = b22
    params[:, PS2V] = np.concatenate(
        [np.full(64, 1.0 / d1, f32), np.full(64, 1.0 / d2, f32)])
    params[:, PBS2] = pairc(b13 + b21)
    params[:, PB13] = pairc(b13)
    params[:, PB23F] = b23
    params[:, PS1] = 0.0

    scal = {
        "s3x4": 4.0 * s3,
        "fast_sign2": bool(np.all(b13 + b21 == 0.0) and np.all(a1 > 0)),
        "sign1_bitwise": bool(np.all(b11 == 0.0)),
        "sign2_bitwise": bool(np.all(b12 == 0.0)),
        "has_b13": bool(np.any(b13 != 0.0)),
        "has_b23": bool(np.any(b23 != 0.0)),
    }
    return wdr8, w2full, params, scal


def kernel(**inputs):
    x = np.ascontiguousarray(np.asarray(inputs["x"], np.float32))
    wdr8, w2full, params, scal = _prep(inputs)

    key = tuple(sorted((k, v) for k, v in scal.items())) + (
        float(params.sum()),)
    if key not in _cache:
        _cache.clear()
        _cache[key] = _build(scal)
    nc, xn, wdrn, w2n, pn, yn = _cache[key]

    in_maps = []
    for i in range(NCORES):
        in_maps.append({
            xn: np.ascontiguousarray(x[BPC * i:BPC * (i + 1)]),
            wdrn: wdr8,
            w2n: w2full,
            pn: params,
        })
    res = bass_utils.run_bass_kernel_spmd(nc, in_maps,
                                          core_ids=list(range(NCORES)))
    out = np.concatenate(
        [res.results[i][yn].astype(np.float32) for i in range(NCORES)],
        axis=0)
    return out


# revision 19
# speedup vs baseline: 238.8927x; 1.0015x over previous
"""Trainium2 Bass kernel for the binarized BasicBlock (dense_cnn).

Contract: kernel(**inputs) takes the FULL unsharded inputs (numpy arrays,
keyed as in reference.setup_inputs()) and returns the FULL output
(32, 128, 56, 56) float32.  Internally shards the batch dim across 8
NeuronCores (pure data parallel, params replicated).

Per-core layout: 4 images as 2 pairs; each pair in 2 half-height units of
28 output rows.  Partitions hold (imgA ch0-63 | imgB ch0-63).  The binary
3x3 conv runs as fp8 DoubleRow matmuls: the stride-2 conv reads adjacent
sign-slab columns (2ox+kx, 2ox+kx+1), so taps (ky,0)+(ky,1) pair naturally
as the two DoubleRow K-planes and (ky,2) pairs with a zero plane -- 6 DR
matmuls per 7-row chunk compute both images into one PSUM tile via
block-diagonal weights.  The avgpool shortcut stays exact fp32 (DVE/GPSIMD
adds) so sign2 never flips vs the reference; sign1 is split between ACT
(AF.Sign -> fp8) and DVE (u32 bitwise trick + fp8 cast).  Stage-2 is the
proven bf16 path: [wpw1|wpw2] pw matmul + diag residual injection into
PSUM, PReLU with per-partition rounding-compensated scales.  Outputs are
stored bf16 and upcast on the host, halving store traffic.
"""
import sys

sys.path.insert(0, "/opt/trn_rl_repo")

import numpy as np
import ml_dtypes

import concourse.bacc as bacc
import concourse.mybir as mybir
import concourse.tile as tile
from concourse import bass_utils

# Problem shapes (hardcoded per spec)
B, CIN, H, W = 32, 64, 112, 112
COUT = 2 * CIN
NCORES = 8
BPC = B // NCORES          # images per core = 4
NPAIR = BPC // 2           # image pairs per core = 2
OH, OW = H // 2, W // 2    # 56, 56
HALF = OH // 2             # 28 output rows per unit
NCHUNK = 4                 # psum chunks per unit (7 out rows each)
CROWS = HALF // NCHUNK     # 7
CN = CROWS * OW            # 392 cols per chunk
UN = HALF * OW             # 1568 elems per unit (per partition)
SROWS = 57                 # raw/sign slab rows (input rows 2*oy0-1 .. 2*oy0+55)
SPITCH = 114               # sign slab col pitch (1 left pad + 112 + 1 right pad)
NA_ACT = 38                # sign1 slab rows on ACT; rest on DVE (balance)

# param columns
PA1, PB12, PB11, PA2F, PB22F, PS2V, PBS2, PB13, PB23F, PS1 = range(10)
NPARAM = 10
# fp8 DoubleRow conv weight blocks: 6 blocks of [128, 2*128]
#   blocks 0..2: planes (ky,0),(ky,1) for ky=0,1,2
#   blocks 3..5: planes (ky,2),zero  for ky=0,1,2
NDR = 6
DRW = 2 * 128
# bf16 stage-2 weights: [wpw1|wpw2] then [diag1|diag2]
O_PW = 0
O_DIAG = 128
W2COLS = 256

_cache = {}


def _build(scal, reps=1):
    """Build the bass program. scal: host-derived scalars/flags.
    reps>1 replicates the whole compute (for slope-based device timing)."""
    nc = bacc.Bacc("TRN2", target_bir_lowering=False, debug=False)
    f32 = mybir.dt.float32
    bf16 = mybir.dt.bfloat16
    fp8 = mybir.dt.float8e4
    u32 = mybir.dt.uint32
    AF = mybir.ActivationFunctionType
    ALU = mybir.AluOpType

    fast_sign2 = scal["fast_sign2"]
    sign1_bitwise = scal["sign1_bitwise"]
    sign2_bitwise = scal["sign2_bitwise"]
    has_b13 = scal["has_b13"]
    has_b23 = scal["has_b23"]

    tc_cm = tile.TileContext(nc)
    tc = tc_cm.__enter__()
    dram_cm = tc.tile_pool(name="dram", bufs=1, space="DRAM")
    dram = dram_cm.__enter__()

    x_d = dram.tile([BPC, CIN, H, W], f32, kind="ExternalInput")
    wdr_d = dram.tile([128, NDR * DRW], fp8, kind="ExternalInput")
    w2_d = dram.tile([128, W2COLS], bf16, kind="ExternalInput")
    p_d = dram.tile([128, NPARAM], f32, kind="ExternalInput")
    y_d = dram.tile([BPC, COUT, OH, OW], bf16, kind="ExternalOutput")

    pools = []

    def pool(name, **kw):
        cm = tc.tile_pool(name=name, **kw)
        pools.append(cm)
        return cm.__enter__()

    const = pool("const", bufs=1)
    pers = pool("pers", bufs=1)
    work = pool("work", bufs=2)
    work1 = pool("work1", bufs=1)
    psum = pool("psum", bufs=4, space="PSUM")
    psum2 = pool("psum2", bufs=2, space="PSUM")

    wdr = const.tile([128, NDR * DRW], fp8)
    w2 = const.tile([128, W2COLS], bf16)
    pt = const.tile([128, NPARAM], f32)
    nc.sync.dma_start(wdr[:], wdr_d[:])
    nc.sync.dma_start(w2[:], w2_d[:])
    nc.sync.dma_start(pt[:], p_d[:])

    # persistent slabs: index by half h (stable pad semantics per buffer)
    xp = [pers.tile([128, SROWS * W], f32, tag=f"xp{h}", name=f"xp{h}")
          for h in range(2)]
    sp = [pers.tile([128, SROWS * SPITCH], fp8, tag=f"sp{h}", name=f"sp{h}")
          for h in range(2)]
    for h in range(2):
        # zero only the pad borders (row 0, col 0, cols 112..113 are data or
        # right pad; col 113 is the DoubleRow zero-plane read for ox=55)
        spv0 = sp[h][:].rearrange("p (r c) -> p r c", r=SROWS)
        nc.vector.memset(spv0[:, 0:1, :], 0.0)
        nc.vector.memset(spv0[:, :, 0:1], 0.0)
        nc.vector.memset(spv0[:, :, 113:114], 0.0)

    def drw(blk):
        # DoubleRow lhsT view for block blk: [128, 2, 128]
        return wdr[:, DRW * blk:DRW * blk + DRW].rearrange(
            "p (two m) -> p two m", two=2)

    units = [(p, h) for _ in range(reps)
             for p in range(NPAIR) for h in range(2)]
    s4s = {}

    BANDS = [(0, 15), (15, 29), (29, 43), (43, SROWS)]

    def _geom(k):
        p, h = units[k]
        r0 = 2 * HALF * h - 1      # input row of slab row 0
        ld0 = 1 if h == 0 else 0   # first valid slab row
        return 2 * p, h, r0, ld0

    def emit_dma(k):
        """x band loads for unit k (2 units ahead of compute)."""
        if k >= len(units):
            return
        nA, h, r0, ld0 = _geom(k)
        xpv = xp[h][:].rearrange("p (r c) -> p r c", r=SROWS)
        for (ra, rb) in BANDS:
            ra = max(ra, ld0)
            src = x_d[nA:nA + 2, :, r0 + ra:r0 + rb, :].rearrange(
                "i c r w -> (i c) r w")
            nc.sync.dma_start(xpv[:, ra:rb, :], src)

    def emit_a(k):
        """Phase A of unit k: sign1 -> sp, avgpool -> s4, per band."""
        if k >= len(units):
            return
        nA, h, r0, ld0 = _geom(k)
        xpv = xp[h][:].rearrange("p (r c) -> p r c", r=SROWS)
        spv = sp[h][:].rearrange("p (r c) -> p r c", r=SROWS)
        na = min(ld0 + NA_ACT, SROWS)
        prow = work1.tile([128, HALF * W], f32, tag="prow", name="prow")
        prv = prow[:].rearrange("p (r c) -> p r c", r=HALF)
        s1f = work1.tile([128, SROWS * W], f32, tag="s1f", name="s1f")
        s1v = s1f[:].rearrange("p (r c) -> p r c", r=SROWS)
        s4 = work.tile([128, UN], f32, tag="s4", name="s4")
        s4v = s4[:].rearrange("p (r c) -> p r c", r=HALF)
        for b, (ra, rb) in enumerate(BANDS):
            ra = max(ra, ld0)
            # sign1 for this band
            if sign1_bitwise and k > 0:
                aa, ab = ra, min(rb, na)       # ACT rows
                da, db = max(ra, na), rb       # DVE rows
                if ab > aa:
                    nc.scalar.activation(
                        spv[:, aa:ab, 1:113], xpv[:, aa:ab, :], AF.Sign)
                if db > da:
                    flat = slice(da * W, db * W)
                    nc.vector.tensor_scalar(
                        s1f[:, flat].bitcast(u32),
                        xp[h][:, flat].bitcast(u32),
                        0x80000000, 0x3F800000,
                        ALU.bitwise_and, ALU.bitwise_or)
                    nc.vector.tensor_copy(spv[:, da:db, 1:113],
                                          s1v[:, da:db, :])
            else:
                nc.scalar.activation(
                    spv[:, ra:rb, 1:113], xpv[:, ra:rb, :],
                    AF.Sign, bias=pt[:, PB11:PB11 + 1])
            # avgpool quarter: prow rows [7b, 7b+7) need xp rows
            # [14b+1, 14b+15) which this band covers
            p0, p1 = 7 * b, 7 * b + 7
            nc.gpsimd.tensor_tensor(
                prv[:, p0:p1, :], xpv[:, 2 * p0 + 1:2 * p1:2, :],
                xpv[:, 2 * p0 + 2:2 * p1 + 1:2, :], ALU.add)
            nc.gpsimd.tensor_tensor(
                s4v[:, p0:p1, :], prv[:, p0:p1, 0:W:2],
                prv[:, p0:p1, 1:W:2], ALU.add)
        s4s[k] = s4

    emit_dma(0)
    emit_dma(1)
    emit_a(0)
    for k, (p, h) in enumerate(units):
        nA, nB = 2 * p, 2 * p + 1
        oy0 = HALF * h
        s4 = s4s.pop(k)
        spv = sp[h][:].rearrange("p (r c) -> p r c", r=SROWS)
        # 2-deep DMA prefetch (xp[h] readers finish early; only sp[h] is
        # read until unit end), 1-deep for sign1/pool of the next unit
        emit_dma(k + 2)
        emit_a(k + 1)

        # ---- fully chunk-pipelined main body: conv_c -> stt_c -> prelu1_c
        # -> sign2_c -> stage2_c -> prelu2_c, so consecutive chunks overlap
        # across PE/DVE/ACT and the PE stream stays dense ----
        u = work.tile([128, UN], f32, tag="u", name="u")
        out1 = work.tile([128, UN], bf16, tag="out1", name="out1")
        sg2 = work.tile([128, UN], bf16, tag="sg2", name="sg2")
        sg2f = work.tile([128, UN], f32, tag="sg2f", name="sg2f")
        stg = [work.tile([128, UN], bf16, tag=f"stg{i}", name=f"stg{i}")
               for i in range(2)]
        fused = fast_sign2 and not has_b13
        for c in range(NCHUNK):
            cp = psum.tile([128, CN], f32, tag="cps", name="cps")
            for t in range(NDR):
                ky = t % 3
                kx0 = 0 if t < 3 else 2
                rs = ky + 14 * c
                rhs = spv[:, rs:rs + 13:2, kx0:kx0 + 112].rearrange(
                    "p r (ox two) -> p two r ox", two=2)
                nc.tensor.matmul(
                    cp[:], drw(t), rhs,
                    start=(t == 0), stop=(t == NDR - 1),
                    perf_mode=mybir.MatmulPerfMode.DoubleRow,
                )
            # u_c = 4*s3*conv + S4  (fused scalar_tensor_tensor on DVE)
            cs = slice(CN * c, CN * (c + 1))
            nc.vector.scalar_tensor_tensor(
                u[:, cs], cp[:], scal["s3x4"], s4[:, cs],
                ALU.mult, ALU.add)
            nc.scalar.activation(
                out1[:, cs], u[:, cs], AF.Prelu,
                bias=pt[:, PB12:PB12 + 1], scale=0.25,
                alpha=pt[:, PA1:PA1 + 1])
            if fused:
                if sign2_bitwise:
                    # sign(0.25*u + b12) with b12==0: u32 bitwise trick on
                    # DVE (ACT is the bottleneck engine), then cast to bf16
                    nc.vector.tensor_scalar(
                        sg2f[:, cs].bitcast(u32), u[:, cs].bitcast(u32),
                        0x80000000, 0x3F800000,
                        ALU.bitwise_and, ALU.bitwise_or)
                    nc.vector.tensor_copy(sg2[:, cs], sg2f[:, cs])
                else:
                    nc.scalar.activation(
                        sg2[:, cs], u[:, cs], AF.Sign,
                        bias=pt[:, PB12:PB12 + 1], scale=0.25)
                for i in range(2):
                    pr = slice(64 * i, 64 * i + 64)
                    cp2 = psum2.tile([128, CN], f32, tag=f"ps{i}",
                                     name=f"ps{i}")
                    nc.tensor.matmul(
                        cp2[:], w2[pr, O_PW:O_PW + 128], sg2[pr, cs],
                        start=True, stop=False)
                    nc.tensor.matmul(
                        cp2[:], w2[pr, O_DIAG:O_DIAG + 128], out1[pr, cs],
                        start=False, stop=True)
                    nc.scalar.activation(
                        stg[i][:, cs], cp2[:], AF.Prelu,
                        bias=pt[:, PB22F:PB22F + 1],
                        scale=pt[:, PS2V:PS2V + 1],
                        alpha=pt[:, PA2F:PA2F + 1])


        if not fused:
            # general fallback (nonzero b13/b21 or non-positive alpha):
            # unit-level sign2/stage2 as in the baseline kernel
            if has_b13:
                nc.vector.tensor_scalar(
                    out1[:], out1[:], pt[:, PB13:PB13 + 1], None, ALU.add)
            if fast_sign2:
                nc.scalar.activation(
                    sg2[:], u[:], AF.Sign,
                    bias=pt[:, PB12:PB12 + 1], scale=0.25)
            else:
                nc.scalar.activation(
                    sg2[:], out1[:], AF.Sign, bias=pt[:, PBS2:PBS2 + 1])
            for i, n in enumerate((nA, nB)):
                pr = slice(64 * i, 64 * i + 64)
                for c in range(NCHUNK):
                    cp2 = psum2.tile([128, CN], f32, tag=f"ps{i}",
                                     name=f"ps{i}")
                    cs = slice(CN * c, CN * (c + 1))
                    nc.tensor.matmul(
                        cp2[:], w2[pr, O_PW:O_PW + 128], sg2[pr, cs],
                        start=True, stop=False)
                    nc.tensor.matmul(
                        cp2[:], w2[pr, O_DIAG:O_DIAG + 128], out1[pr, cs],
                        start=False, stop=True)
                    nc.scalar.activation(
                        stg[i][:, cs], cp2[:], AF.Prelu,
                        bias=pt[:, PB22F:PB22F + 1],
                        scale=pt[:, PS2V:PS2V + 1],
                        alpha=pt[:, PA2F:PA2F + 1])
                if has_b23:
                    nc.vector.tensor_scalar(
                        stg[i][:], stg[i][:], pt[:, PB23F:PB23F + 1],
                        None, ALU.add)

        # ---- store (bf16): two 128-partition DMAs per image ----
        for i, n in enumerate((nA, nB)):
            sv = stg[i][:].rearrange("p (r c) -> p r c", r=HALF)
            hh = HALF // 2
            nc.sync.dma_start(y_d[n, :, oy0:oy0 + hh, :], sv[:, 0:hh, :])
            nc.sync.dma_start(y_d[n, :, oy0 + hh:oy0 + HALF, :],
                              sv[:, hh:HALF, :])

    for cm in reversed(pools):
        cm.__exit__(None, None, None)
    dram_cm.__exit__(None, None, None)
    tc_cm.__exit__(None, None, None)
    nc.compile()
    return nc, x_d.name, wdr_d.name, w2_d.name, p_d.name, y_d.name


def _prep(inputs):
    f32 = np.float32
    bf = ml_dtypes.bfloat16
    f8 = ml_dtypes.float8_e4m3fn
    w3 = np.asarray(inputs["w3"], f32)
    wpw1 = np.asarray(inputs["wpw1"], f32)
    wpw2 = np.asarray(inputs["wpw2"], f32)
    a1 = np.asarray(inputs["a1"], f32).reshape(CIN)
    a2 = np.asarray(inputs["a2"], f32).reshape(COUT)
    b11 = np.asarray(inputs["b11"], f32).reshape(CIN)
    b12 = np.asarray(inputs["b12"], f32).reshape(CIN)
    b13 = np.asarray(inputs["b13"], f32).reshape(CIN)
    b21 = np.asarray(inputs["b21"], f32).reshape(CIN)
    b22 = np.asarray(inputs["b22"], f32).reshape(COUT)
    b23 = np.asarray(inputs["b23"], f32).reshape(COUT)

    s3 = float(np.mean(np.abs(w3))) or 1.0
    s1 = float(np.mean(np.abs(wpw1))) or 1.0
    s2 = float(np.mean(np.abs(wpw2))) or 1.0

    # diag entries bf16(1/s_j); prelu2 scale 1/d_j compensates the rounding
    d1 = float(bf(1.0 / s1))
    d2 = float(bf(1.0 / s2))

    sgn = np.sign
    # fp8 DoubleRow conv weights: 6 blocks [128, 2, 128] block-diagonal over
    # the two images; plane order matches the rhs (ox two) factorization
    wdr = np.zeros((128, NDR, 2, 128), f32)
    for t in range(NDR):
        ky = t % 3
        kx0 = 0 if t < 3 else 2
        for i, kx in enumerate((kx0, kx0 + 1)):
            if kx > 2:
                continue  # zero plane
            wt = sgn(w3[:, :, ky, kx]).T       # [k=cin, m=cout]
            wdr[0:64, t, i, 0:64] = wt
            wdr[64:128, t, i, 64:128] = wt
    wdr8 = np.ascontiguousarray(
        wdr.reshape(128, NDR * DRW)).astype(f8)

    w2half = np.zeros((64, W2COLS), f32)
    w2half[:, O_PW:O_PW + 64] = sgn(wpw1[:, :, 0, 0]).T
    w2half[:, O_PW + 64:O_PW + 128] = sgn(wpw2[:, :, 0, 0]).T
    w2half[:, O_DIAG:O_DIAG + 64] = d1 * np.eye(64, dtype=f32)
    w2half[:, O_DIAG + 64:O_DIAG + 128] = d2 * np.eye(64, dtype=f32)
    w2full = np.concatenate([w2half, w2half], axis=0).astype(bf)

    def pairc(v):  # channel vec (64,) -> pair-layout (128,)
        return np.concatenate([v, v])

    params = np.zeros((128, NPARAM), f32)
    params[:, PA1] = pairc(a1)
    params[:, PB12] = pairc(b12)
    params[:, PB11] = pairc(b11)
    params[:, PA2F] = a2
    params[:, PB22F] = b22
    params[:, PS2V] = np.concatenate(
        [np.full(64, 1.0 / d1, f32), np.full(64, 1.0 / d2, f32)])
    params[:, PBS2] = pairc(b13 + b21)
    params[:, PB13] = pairc(b13)
    params[:, PB23F] = b23
    params[:, PS1] = 0.0

    scal = {
        "s3x4": 4.0 * s3,
        "fast_sign2": bool(np.all(b13 + b21 == 0.0) and np.all(a1 > 0)),
        "sign1_bitwise": bool(np.all(b11 == 0.0)),
        "sign2_bitwise": bool(np.all(b12 == 0.0)),
        "has_b13": bool(np.any(b13 != 0.0)),
        "has_b23": bool(np.any(b23 != 0.0)),
    }
    return wdr8, w2full, params, scal


def kernel(**inputs):
    x = np.ascontiguousarray(np.asarray(inputs["x"], np.float32))
    wdr8, w2full, params, scal = _prep(inputs)

    key = tuple(sorted((k, v) for k, v in scal.items())) + (
        float(params.sum()),)
    if key not in _cache:
        _cache.clear()
        _cache[key] = _build(scal)
    nc, xn, wdrn, w2n, pn, yn = _cache[key]

    in_maps = []
    for i in range(NCORES):
        in_maps.append({
            xn: np.ascontiguousarray(x[BPC * i:BPC * (i + 1)]),
            wdrn: wdr8,
            w2n: w2full,
            pn: params,
        })
    res = bass_utils.run_bass_kernel_spmd(nc, in_maps,
                                          core_ids=list(range(NCORES)))
    out = np.concatenate(
        [res.results[i][yn].astype(np.float32) for i in range(NCORES)],
        axis=0)
    return out


# revision 29
# speedup vs baseline: 260.3408x; 1.0898x over previous
"""Trainium2 Bass kernel for the binarized BasicBlock (dense_cnn).

Contract: kernel(**inputs) takes the FULL unsharded inputs (numpy arrays,
keyed as in reference.setup_inputs()) and returns the FULL output
(32, 128, 56, 56) float32.  Internally shards the batch dim across 8
NeuronCores (pure data parallel, params replicated).

Per-core layout: 4 images as 2 pairs; each pair in 2 half-height units of
28 output rows.  Partitions hold (imgA ch0-63 | imgB ch0-63).  The binary
3x3 conv runs as fp8 DoubleRow matmuls: the stride-2 conv reads adjacent
sign-slab columns (2ox+kx, 2ox+kx+1), so taps (ky,0)+(ky,1) pair naturally
as the two DoubleRow K-planes and (ky,2) pairs with a zero plane -- 6 DR
matmuls per 7-row chunk compute both images into one PSUM tile via
block-diagonal weights.  The avgpool shortcut stays exact fp32 (DVE/GPSIMD
adds) so sign2 never flips vs the reference; sign1 is split between ACT
(AF.Sign -> fp8) and DVE (u32 bitwise trick + fp8 cast).  Stage-2 is the
proven bf16 path: [wpw1|wpw2] pw matmul + diag residual injection into
PSUM, PReLU with per-partition rounding-compensated scales.  Outputs are
stored bf16 and upcast on the host, halving store traffic.
"""
import sys

sys.path.insert(0, "/opt/trn_rl_repo")

import numpy as np
import ml_dtypes

import concourse.bacc as bacc
import concourse.mybir as mybir
import concourse.tile as tile
from concourse import bass_utils

# Problem shapes (hardcoded per spec)
B, CIN, H, W = 32, 64, 112, 112
COUT = 2 * CIN
NCORES = 8
BPC = B // NCORES          # images per core = 4
NPAIR = BPC // 2           # image pairs per core = 2
OH, OW = H // 2, W // 2    # 56, 56
HALF = OH // 2             # 28 output rows per unit
NCHUNK = 4                 # psum chunks per unit (7 out rows each)
CROWS = HALF // NCHUNK     # 7
CN = CROWS * OW            # 392 cols per chunk
UN = HALF * OW             # 1568 elems per unit (per partition)
SROWS = 57                 # raw/sign slab rows (input rows 2*oy0-1 .. 2*oy0+55)
SPITCH = 114               # sign slab col pitch (1 left pad + 112 + 1 right pad)
NA_ACT = 28                # sign1 slab rows on ACT; rest on DVE (balance)

# param columns
PA1, PB12, PB11, PA2F, PB22F, PS2V, PBS2, PB13, PB23F, PS1 = range(10)
NPARAM = 10
# fp8 DoubleRow conv weight blocks: 6 blocks of [128, 2*128]
#   blocks 0..2: planes (ky,0),(ky,1) for ky=0,1,2
#   blocks 3..5: planes (ky,2),zero  for ky=0,1,2
NDR = 6
DRW = 2 * 128
# bf16 stage-2 weights: [wpw1|wpw2] then [diag1|diag2]
O_PW = 0
O_DIAG = 128
W2COLS = 256

_cache = {}


def _build(scal, reps=1):
    """Build the bass program. scal: host-derived scalars/flags.
    reps>1 replicates the whole compute (for slope-based device timing)."""
    nc = bacc.Bacc("TRN2", target_bir_lowering=False, debug=False)
    f32 = mybir.dt.float32
    f16 = mybir.dt.float16
    bf16 = mybir.dt.bfloat16
    fp8 = mybir.dt.float8e4
    u32 = mybir.dt.uint32
    AF = mybir.ActivationFunctionType
    ALU = mybir.AluOpType

    fast_sign2 = scal["fast_sign2"]
    sign1_bitwise = scal["sign1_bitwise"]
    has_b13 = scal["has_b13"]
    has_b23 = scal["has_b23"]

    tc_cm = tile.TileContext(nc)
    tc = tc_cm.__enter__()
    dram_cm = tc.tile_pool(name="dram", bufs=1, space="DRAM")
    dram = dram_cm.__enter__()

    x_d = dram.tile([BPC, CIN, H, W], f16, kind="ExternalInput")
    wdr_d = dram.tile([128, NDR * DRW], fp8, kind="ExternalInput")
    w2_d = dram.tile([128, W2COLS], bf16, kind="ExternalInput")
    p_d = dram.tile([128, NPARAM], f32, kind="ExternalInput")
    y_d = dram.tile([BPC, COUT, OH, OW], bf16, kind="ExternalOutput")

    pools = []

    def pool(name, **kw):
        cm = tc.tile_pool(name=name, **kw)
        pools.append(cm)
        return cm.__enter__()

    const = pool("const", bufs=1)
    pers = pool("pers", bufs=1)
    work = pool("work", bufs=2)
    work1 = pool("work1", bufs=1)
    psum = pool("psum", bufs=4, space="PSUM")
    psum2 = pool("psum2", bufs=2, space="PSUM")

    wdr = const.tile([128, NDR * DRW], fp8)
    w2 = const.tile([128, W2COLS], bf16)
    pt = const.tile([128, NPARAM], f32)
    nc.sync.dma_start(wdr[:], wdr_d[:])
    nc.sync.dma_start(w2[:], w2_d[:])
    nc.sync.dma_start(pt[:], p_d[:])

    # persistent slabs: index by half h (stable pad semantics per buffer)
    xp = [pers.tile([128, SROWS * W], f16, tag=f"xp{h}", name=f"xp{h}")
          for h in range(2)]
    sp = [pers.tile([128, SROWS * SPITCH], fp8, tag=f"sp{h}", name=f"sp{h}")
          for h in range(2)]
    for h in range(2):
        # zero only the pad borders (row 0, col 0, cols 112..113 are data or
        # right pad; col 113 is the DoubleRow zero-plane read for ox=55)
        spv0 = sp[h][:].rearrange("p (r c) -> p r c", r=SROWS)
        nc.vector.memset(spv0[:, 0:1, :], 0.0)
        nc.vector.memset(spv0[:, :, 0:1], 0.0)
        nc.vector.memset(spv0[:, :, 113:114], 0.0)

    def drw(blk):
        # DoubleRow lhsT view for block blk: [128, 2, 128]
        return wdr[:, DRW * blk:DRW * blk + DRW].rearrange(
            "p (two m) -> p two m", two=2)

    units = [(p, h) for _ in range(reps)
             for p in range(NPAIR) for h in range(2)]
    s4s = {}

    BANDS = [(0, 15), (15, 29), (29, 43), (43, SROWS)]

    def _geom(k):
        p, h = units[k]
        r0 = 2 * HALF * h - 1      # input row of slab row 0
        ld0 = 1 if h == 0 else 0   # first valid slab row
        return 2 * p, h, r0, ld0

    def emit_dma(k):
        """x band loads for unit k (2 units ahead of compute)."""
        if k >= len(units):
            return
        nA, h, r0, ld0 = _geom(k)
        xpv = xp[h][:].rearrange("p (r c) -> p r c", r=SROWS)
        for (ra, rb) in BANDS:
            ra = max(ra, ld0)
            src = x_d[nA:nA + 2, :, r0 + ra:r0 + rb, :].rearrange(
                "i c r w -> (i c) r w")
            nc.sync.dma_start(xpv[:, ra:rb, :], src)

    def emit_a(k):
        """Phase A of unit k: sign1 -> sp, avgpool -> s4, per band."""
        if k >= len(units):
            return
        nA, h, r0, ld0 = _geom(k)
        xpv = xp[h][:].rearrange("p (r c) -> p r c", r=SROWS)
        spv = sp[h][:].rearrange("p (r c) -> p r c", r=SROWS)
        na = min(ld0 + NA_ACT, SROWS)
        prow = work1.tile([128, HALF * W], f32, tag="prow", name="prow")
        prv = prow[:].rearrange("p (r c) -> p r c", r=HALF)
        s1f = work1.tile([128, SROWS * W], f16, tag="s1f", name="s1f")
        s1v = s1f[:].rearrange("p (r c) -> p r c", r=SROWS)
        s4 = work.tile([128, UN], f32, tag="s4", name="s4")
        s4v = s4[:].rearrange("p (r c) -> p r c", r=HALF)
        for b, (ra, rb) in enumerate(BANDS):
            ra = max(ra, ld0)
            # sign1 for this band
            if sign1_bitwise and k > 0:
                aa, ab = ra, min(rb, na)       # ACT rows
                da, db = max(ra, na), rb       # DVE rows
                if ab > aa:
                    nc.scalar.activation(
                        spv[:, aa:ab, 1:113], xpv[:, aa:ab, :], AF.Sign)
                if db > da:
                    # u32 bitwise over fp16 PAIRS: two signs per lane-op
                    flat = slice(da * W, db * W)
                    nc.vector.tensor_scalar(
                        s1f[:, flat].bitcast(u32),
                        xp[h][:, flat].bitcast(u32),
                        0x80008000, 0x3C003C00,
                        ALU.bitwise_and, ALU.bitwise_or)
                    nc.vector.tensor_copy(spv[:, da:db, 1:113],
                                          s1v[:, da:db, :])
            else:
                nc.scalar.activation(
                    spv[:, ra:rb, 1:113], xpv[:, ra:rb, :],
                    AF.Sign, bias=pt[:, PB11:PB11 + 1])
            # avgpool quarter: prow rows [7b, 7b+7) need xp rows
            # [14b+1, 14b+15) which this band covers
            p0, p1 = 7 * b, 7 * b + 7
            nc.gpsimd.tensor_tensor(
                prv[:, p0:p1, :], xpv[:, 2 * p0 + 1:2 * p1:2, :],
                xpv[:, 2 * p0 + 2:2 * p1 + 1:2, :], ALU.add)
            nc.gpsimd.tensor_tensor(
                s4v[:, p0:p1, :], prv[:, p0:p1, 0:W:2],
                prv[:, p0:p1, 1:W:2], ALU.add)
        s4s[k] = s4

    emit_dma(0)
    emit_dma(1)
    emit_a(0)
    for k, (p, h) in enumerate(units):
        nA, nB = 2 * p, 2 * p + 1
        oy0 = HALF * h
        s4 = s4s.pop(k)
        spv = sp[h][:].rearrange("p (r c) -> p r c", r=SROWS)
        # 2-deep DMA prefetch (xp[h] readers finish early; only sp[h] is
        # read until unit end), 1-deep for sign1/pool of the next unit
        emit_dma(k + 2)
        emit_a(k + 1)

        # ---- fully chunk-pipelined main body: conv_c -> stt_c -> prelu1_c
        # -> sign2_c -> stage2_c -> prelu2_c, so consecutive chunks overlap
        # across PE/DVE/ACT and the PE stream stays dense ----
        u = work.tile([128, UN], f32, tag="u", name="u")
        out1 = work.tile([128, UN], bf16, tag="out1", name="out1")
        sg2 = work.tile([128, UN], bf16, tag="sg2", name="sg2")
        stg = [work.tile([128, UN], bf16, tag=f"stg{i}", name=f"stg{i}")
               for i in range(2)]
        fused = fast_sign2 and not has_b13
        for c in range(NCHUNK):
            cp = psum.tile([128, CN], f32, tag="cps", name="cps")
            for t in range(NDR):
                ky = t % 3
                kx0 = 0 if t < 3 else 2
                rs = ky + 14 * c
                rhs = spv[:, rs:rs + 13:2, kx0:kx0 + 112].rearrange(
                    "p r (ox two) -> p two r ox", two=2)
                nc.tensor.matmul(
                    cp[:], drw(t), rhs,
                    start=(t == 0), stop=(t == NDR - 1),
                    perf_mode=mybir.MatmulPerfMode.DoubleRow,
                )
            # u_c = 4*s3*conv + S4  (fused scalar_tensor_tensor on DVE)
            cs = slice(CN * c, CN * (c + 1))
            nc.vector.scalar_tensor_tensor(
                u[:, cs], cp[:], scal["s3x4"], s4[:, cs],
                ALU.mult, ALU.add)
            nc.scalar.activation(
                out1[:, cs], u[:, cs], AF.Prelu,
                bias=pt[:, PB12:PB12 + 1], scale=0.25,
                alpha=pt[:, PA1:PA1 + 1])
            if fused:
                # sg2 = sign(out1) (alpha>0 makes prelu sign-preserving and
                # b12 is inside out1): ONE u32 bitwise pass over bf16 pairs
                nc.vector.tensor_scalar(
                    sg2[:, cs].bitcast(u32), out1[:, cs].bitcast(u32),
                    0x80008000, 0x3F803F80,
                    ALU.bitwise_and, ALU.bitwise_or)
                for i in range(2):
                    pr = slice(64 * i, 64 * i + 64)
                    cp2 = psum2.tile([128, CN], f32, tag=f"ps{i}",
                                     name=f"ps{i}")
                    nc.tensor.matmul(
                        cp2[:], w2[pr, O_PW:O_PW + 128], sg2[pr, cs],
                        start=True, stop=False)
                    nc.tensor.matmul(
                        cp2[:], w2[pr, O_DIAG:O_DIAG + 128], out1[pr, cs],
                        start=False, stop=True)
                    nc.scalar.activation(
                        stg[i][:, cs], cp2[:], AF.Prelu,
                        bias=pt[:, PB22F:PB22F + 1],
                        scale=pt[:, PS2V:PS2V + 1],
                        alpha=pt[:, PA2F:PA2F + 1])


        if not fused:
            # general fallback (nonzero b13/b21 or non-positive alpha):
            # unit-level sign2/stage2 as in the baseline kernel
            if has_b13:
                nc.vector.tensor_scalar(
                    out1[:], out1[:], pt[:, PB13:PB13 + 1], None, ALU.add)
            if fast_sign2:
                nc.scalar.activation(
                    sg2[:], u[:], AF.Sign,
                    bias=pt[:, PB12:PB12 + 1], scale=0.25)
            else:
                nc.scalar.activation(
                    sg2[:], out1[:], AF.Sign, bias=pt[:, PBS2:PBS2 + 1])
            for i, n in enumerate((nA, nB)):
                pr = slice(64 * i, 64 * i + 64)
                for c in range(NCHUNK):
                    cp2 = psum2.tile([128, CN], f32, tag=f"ps{i}",
                                     name=f"ps{i}")
                    cs = slice(CN * c, CN * (c + 1))
                    nc.tensor.matmul(
                        cp2[:], w2[pr, O_PW:O_PW + 128], sg2[pr, cs],
                        start=True, stop=False)
                    nc.tensor.matmul(
                        cp2[:], w2[pr, O_DIAG:O_DIAG + 128], out1[pr, cs],
                        start=False, stop=True)
                    nc.scalar.activation(
                        stg[i][:, cs], cp2[:], AF.Prelu,
                        bias=pt[:, PB22F:PB22F + 1],
                        scale=pt[:, PS2V:PS2V + 1],
                        alpha=pt[:, PA2F:PA2F + 1])
                if has_b23:
                    nc.vector.tensor_scalar(
                        stg[i][:], stg[i][:], pt[:, PB23F:PB23F + 1],
                        None, ALU.add)

        # ---- store (bf16): two 128-partition DMAs per image ----
        for i, n in enumerate((nA, nB)):
            sv = stg[i][:].rearrange("p (r c) -> p r c", r=HALF)
            hh = HALF // 2
            nc.sync.dma_start(y_d[n, :, oy0:oy0 + hh, :], sv[:, 0:hh, :])
            nc.sync.dma_start(y_d[n, :, oy0 + hh:oy0 + HALF, :],
                              sv[:, hh:HALF, :])

    for cm in reversed(pools):
        cm.__exit__(None, None, None)
    dram_cm.__exit__(None, None, None)
    tc_cm.__exit__(None, None, None)
    nc.compile()
    return nc, x_d.name, wdr_d.name, w2_d.name, p_d.name, y_d.name


def _prep(inputs):
    f32 = np.float32
    bf = ml_dtypes.bfloat16
    f8 = ml_dtypes.float8_e4m3fn
    w3 = np.asarray(inputs["w3"], f32)
    wpw1 = np.asarray(inputs["wpw1"], f32)
    wpw2 = np.asarray(inputs["wpw2"], f32)
    a1 = np.asarray(inputs["a1"], f32).reshape(CIN)
    a2 = np.asarray(inputs["a2"], f32).reshape(COUT)
    b11 = np.asarray(inputs["b11"], f32).reshape(CIN)
    b12 = np.asarray(inputs["b12"], f32).reshape(CIN)
    b13 = np.asarray(inputs["b13"], f32).reshape(CIN)
    b21 = np.asarray(inputs["b21"], f32).reshape(CIN)
    b22 = np.asarray(inputs["b22"], f32).reshape(COUT)
    b23 = np.asarray(inputs["b23"], f32).reshape(COUT)

    s3 = float(np.mean(np.abs(w3))) or 1.0
    s1 = float(np.mean(np.abs(wpw1))) or 1.0
    s2 = float(np.mean(np.abs(wpw2))) or 1.0

    # diag entries bf16(1/s_j); prelu2 scale 1/d_j compensates the rounding
    d1 = float(bf(1.0 / s1))
    d2 = float(bf(1.0 / s2))

    sgn = np.sign
    # fp8 DoubleRow conv weights: 6 blocks [128, 2, 128] block-diagonal over
    # the two images; plane order matches the rhs (ox two) factorization
    wdr = np.zeros((128, NDR, 2, 128), f32)
    for t in range(NDR):
        ky = t % 3
        kx0 = 0 if t < 3 else 2
        for i, kx in enumerate((kx0, kx0 + 1)):
            if kx > 2:
                continue  # zero plane
            wt = sgn(w3[:, :, ky, kx]).T       # [k=cin, m=cout]
            wdr[0:64, t, i, 0:64] = wt
            wdr[64:128, t, i, 64:128] = wt
    wdr8 = np.ascontiguousarray(
        wdr.reshape(128, NDR * DRW)).astype(f8)

    w2half = np.zeros((64, W2COLS), f32)
    w2half[:, O_PW:O_PW + 64] = sgn(wpw1[:, :, 0, 0]).T
    w2half[:, O_PW + 64:O_PW + 128] = sgn(wpw2[:, :, 0, 0]).T
    w2half[:, O_DIAG:O_DIAG + 64] = d1 * np.eye(64, dtype=f32)
    w2half[:, O_DIAG + 64:O_DIAG + 128] = d2 * np.eye(64, dtype=f32)
    w2full = np.concatenate([w2half, w2half], axis=0).astype(bf)

    def pairc(v):  # channel vec (64,) -> pair-layout (128,)
        return np.concatenate([v, v])

    params = np.zeros((128, NPARAM), f32)
    params[:, PA1] = pairc(a1)
    params[:, PB12] = pairc(b12)
    params[:, PB11] = pairc(b11)
    params[:, PA2F] = a2
    params[:, PB22F] = b22
    params[:, PS2V] = np.concatenate(
        [np.full(64, 1.0 / d1, f32), np.full(64, 1.0 / d2, f32)])
    params[:, PBS2] = pairc(b13 + b21)
    params[:, PB13] = pairc(b13)
    params[:, PB23F] = b23
    params[:, PS1] = 0.0

    scal = {
        "s3x4": 4.0 * s3,
        "fast_sign2": bool(np.all(b13 + b21 == 0.0) and np.all(a1 > 0)),
        "sign1_bitwise": bool(np.all(b11 == 0.0)),
        "sign2_bitwise": bool(np.all(b12 == 0.0)),
        "has_b13": bool(np.any(b13 != 0.0)),
        "has_b23": bool(np.any(b23 != 0.0)),
    }
    return wdr8, w2full, params, scal


def kernel(**inputs):
    x = np.ascontiguousarray(
        np.asarray(inputs["x"], np.float32).astype(np.float16))
    wdr8, w2full, params, scal = _prep(inputs)

    key = tuple(sorted((k, v) for k, v in scal.items())) + (
        float(params.sum()),)
    if key not in _cache:
        _cache.clear()
        _cache[key] = _build(scal)
    nc, xn, wdrn, w2n, pn, yn = _cache[key]

    in_maps = []
    for i in range(NCORES):
        in_maps.append({
            xn: np.ascontiguousarray(x[BPC * i:BPC * (i + 1)]),
            wdrn: wdr8,
            w2n: w2full,
            pn: params,
        })
    res = bass_utils.run_bass_kernel_spmd(nc, in_maps,
                                          core_ids=list(range(NCORES)))
    out = np.concatenate(
        [res.results[i][yn].astype(np.float32) for i in range(NCORES)],
        axis=0)
    return out
